# revision 7
# baseline (speedup 1.0000x reference)
"""Trainium2 Bass kernel for nn_EquivariantEmbedding (gnn_message_passing).

Sharding: edge-parallel across 8 cores (58500 edges each, padded to 58624);
node embedding data-parallel (2500 nodes/core, padded to 2560). Small
embedding tables are replicated. All compute (gathers, rotation matrices,
gaussian smearing, source/target one-hot embeddings, bit-packing) happens
on device.

Per-edge node data (pos + atomic number, packed 16B rows) is gathered with
SWDGE indirect DMAs, 128 rows per op. Source/target embeddings are computed
on the tensor engine as one-hot(atomic number) @ table matmuls — no 512B-row
gathers needed. Rotation matrices are computed in a structure-of-arrays
layout with full-width vector ops.
"""
import sys

for _p in ("/opt/trn_rl_repo",):
    if _p not in sys.path:
        sys.path.append(_p)

import numpy as np

import concourse.bacc as bacc
import concourse.bass as bass
import concourse.tile as tile
from concourse import mybir
from concourse.bass import IndirectOffsetOnAxis
from concourse.bass_utils import run_bass_kernel_spmd
from concourse.masks import make_identity

F32 = mybir.dt.float32
I32 = mybir.dt.int32

N_CORES = 8
N_NODES = 20000
N_EDGES = 468000
C = 128
NG = 128
NUM_COEF = 16
MAX_ELEM = 90
EMB2 = 32767
CUTOFF = 5.0
BWS = 20.0

E_PER = N_EDGES // N_CORES            # 58500
E_CH = (E_PER + 127) // 128           # 458 chunks of 128 edges
E_PAD = E_CH * 128                    # 58624

NP_PER = N_NODES // N_CORES           # 2500
NP_CH = (NP_PER + 127) // 128         # 20
NP_PAD = NP_CH * 128                  # 2560

SG_CH = 16                            # chunks per supergroup (2048 edges)

_offs = np.linspace(0.0, CUTOFF, NG, dtype=np.float32)
OFF_STEP = float(_offs[1] - _offs[0])
COEFF = float(-0.5 / (BWS * (_offs[1] - _offs[0])) ** 2)

TRACE = False  # test.py sets this for profiled runs
LAST_RESULT = None


def _ap(t, offset, dims):
    return bass.AP(tensor=t.tensor if hasattr(t, "tensor") else t, offset=offset,
                   ap=[list(d) for d in dims])


def build_module():
    nc = bacc.Bacc("TRN2", target_bir_lowering=False, debug=False, num_devices=N_CORES)

    pos = nc.dram_tensor("pos", [N_NODES, 3], F32, kind="ExternalInput")
    sphere_w = nc.dram_tensor("sphere_w", [MAX_ELEM, C], F32, kind="ExternalInput")
    sphere2_w = nc.dram_tensor("sphere2_w", [EMB2, C], F32, kind="ExternalInput")
    source_w = nc.dram_tensor("source_w", [MAX_ELEM, NG], F32, kind="ExternalInput")
    target_w = nc.dram_tensor("target_w", [MAX_ELEM, NG], F32, kind="ExternalInput")
    an_full = nc.dram_tensor("an_full", [N_NODES], I32, kind="ExternalInput")
    src_idx = nc.dram_tensor("src_idx", [E_PAD], I32, kind="ExternalInput")
    tgt_idx = nc.dram_tensor("tgt_idx", [E_PAD], I32, kind="ExternalInput")
    rand = nc.dram_tensor("rand", [E_PAD, 3], F32, kind="ExternalInput")
    an_node = nc.dram_tensor("an_node", [NP_PAD], I32, kind="ExternalInput")
    xbits = nc.dram_tensor("xbits", [NP_PAD, 15], I32, kind="ExternalInput")

    node_out = nc.dram_tensor("node_out", [NP_PAD, NUM_COEF, C], F32, kind="ExternalOutput")
    feat_out = nc.dram_tensor("feat_out", [E_PAD, 3 * NG], F32, kind="ExternalOutput")
    rot_out = nc.dram_tensor("rot_out", [E_PAD, 9], F32, kind="ExternalOutput")

    AF = mybir.ActivationFunctionType
    OP = mybir.AluOpType

    with tile.TileContext(nc) as tc:
        import contextlib
        with contextlib.ExitStack() as ctx:
            consts = ctx.enter_context(tc.tile_pool(name="consts", bufs=1))
            dram = ctx.enter_context(tc.tile_pool(name="dram", bufs=1, space="DRAM"))

            # ---- constants ----
            offs_i = consts.tile([128, NG], I32)
            nc.gpsimd.iota(offs_i[:], pattern=[[1, NG]], base=0, channel_multiplier=0)
            offs_f = consts.tile([128, NG], F32)
            nc.vector.tensor_copy(offs_f[:], offs_i[:])
            nc.vector.tensor_scalar_mul(offs_f[:], offs_f[:], OFF_STEP)

            zero_t = consts.tile([128, (NUM_COEF - 1) * C], F32)
            nc.vector.memset(zero_t[:], 0.0)

            ident = consts.tile([128, 128], F32)
            make_identity(nc, ident[:])

            # iota down partitions (value = partition index) as f32, for one-hots
            iota_p = consts.tile([128, 1], I32)
            nc.gpsimd.iota(iota_p[:], pattern=[[0, 1]], base=0, channel_multiplier=1)
            iota_pf = consts.tile([128, 1], F32)
            nc.vector.tensor_copy(iota_pf[:], iota_p[:])

            # embedding tables resident in SBUF
            srcw_sb = consts.tile([MAX_ELEM, C], F32)
            tgtw_sb = consts.tile([MAX_ELEM, C], F32)
            nc.sync.dma_start(out=srcw_sb[:], in_=source_w[:])
            nc.sync.dma_start(out=tgtw_sb[:], in_=target_w[:])

            # ---- persistent edge-index tiles: idx32[p, ch] = idx[128*ch + p] ----
            s_i32 = consts.tile([128, E_CH], I32, name="s_i32")
            t_i32 = consts.tile([128, E_CH], I32, name="t_i32")
            nc.sync.dma_start(out=s_i32[:], in_=_ap(src_idx, 0, [[1, 128], [128, E_CH]]))
            nc.sync.dma_start(out=t_i32[:], in_=_ap(tgt_idx, 0, [[1, 128], [128, E_CH]]))

            # ---- packed node table [N_NODES, 4]: (x, y, z, float(an)) ----
            packed = dram.tile([N_NODES, 4], F32)
            nc.sync.dma_start(
                out=_ap(packed, packed.offset, [[4, N_NODES], [1, 3]]),
                in_=pos[:],
            )
            NACH = (N_NODES + 127) // 128  # 157
            an_sb = consts.tile([128, NACH], I32, name="an_sb")
            nc.vector.memset(an_sb[:], 0)
            nc.sync.dma_start(
                out=an_sb[:, :NACH - 1],
                in_=_ap(an_full, 0, [[1, 128], [128, NACH - 1]]),
            )
            nc.sync.dma_start(  # tail chunk (32 nodes)
                out=an_sb[0:N_NODES - 128 * (NACH - 1), NACH - 1:NACH],
                in_=_ap(an_full, 128 * (NACH - 1), [[1, N_NODES - 128 * (NACH - 1)], [1, 1]]),
            )
            an_f = consts.tile([128, NACH], F32, name="an_f")
            nc.vector.tensor_copy(an_f[:], an_sb[:])
            nc.sync.dma_start(
                out=_ap(packed, packed.offset + 3, [[4, 128], [512, NACH - 1], [1, 1]]),
                in_=an_f[:, :NACH - 1],
            )
            nc.sync.dma_start(
                out=_ap(packed, packed.offset + 3 + 512 * (NACH - 1),
                        [[4, N_NODES - 128 * (NACH - 1)], [1, 1]]),
                in_=an_f[0:N_NODES - 128 * (NACH - 1), NACH - 1:NACH],
            )

            # ---- SoA tiles for the rotation-matrix stage ----
            evx = consts.tile([128, E_CH], F32, name="evx")
            evy = consts.tile([128, E_CH], F32, name="evy")
            evz = consts.tile([128, E_CH], F32, name="evz")
            d_all = consts.tile([128, E_CH], F32, name="d_all")

            sg_bounds = []
            ch0 = 0
            while ch0 < E_CH:
                sg_bounds.append((ch0, min(SG_CH, E_CH - ch0)))
                ch0 += SG_CH

            with tc.tile_pool(name="sg", bufs=3) as sg, \
                 tc.tile_pool(name="ps", bufs=2, space="PSUM") as ps:
                for (c0, nch) in sg_bounds:
                    e0 = c0 * 128
                    # gather packed rows for src/tgt, one [128,1]-offset op per chunk
                    pk_src = sg.tile([128, SG_CH * 4], F32, tag="psrc")
                    pk_tgt = sg.tile([128, SG_CH * 4], F32, tag="ptgt")
                    for t in range(nch):
                        nc.gpsimd.indirect_dma_start(
                            out=pk_src[:, 4 * t:4 * t + 4], out_offset=None,
                            in_=packed[:],
                            in_offset=IndirectOffsetOnAxis(ap=s_i32[:, c0 + t:c0 + t + 1], axis=0),
                        )
                        nc.gpsimd.indirect_dma_start(
                            out=pk_tgt[:, 4 * t:4 * t + 4], out_offset=None,
                            in_=packed[:],
                            in_offset=IndirectOffsetOnAxis(ap=t_i32[:, c0 + t:c0 + t + 1], axis=0),
                        )
                    # ev components into SoA tiles
                    for comp, dst in ((0, evx), (1, evy), (2, evz)):
                        nc.vector.tensor_tensor(
                            out=dst[:, c0:c0 + nch],
                            in0=_ap(pk_src, pk_src.offset + comp, [pk_src.ap[0], [4, nch]]),
                            in1=_ap(pk_tgt, pk_tgt.offset + comp, [pk_tgt.ap[0], [4, nch]]),
                            op=OP.subtract,
                        )
                    # d = sqrt(evx^2 + evy^2 + evz^2)
                    t1 = sg.tile([128, SG_CH], F32, tag="t1")
                    t2 = sg.tile([128, SG_CH], F32, tag="t2")
                    nc.vector.tensor_mul(t1[:, :nch], evx[:, c0:c0 + nch], evx[:, c0:c0 + nch])
                    nc.vector.tensor_mul(t2[:, :nch], evy[:, c0:c0 + nch], evy[:, c0:c0 + nch])
                    nc.vector.tensor_add(t1[:, :nch], t1[:, :nch], t2[:, :nch])
                    nc.vector.tensor_mul(t2[:, :nch], evz[:, c0:c0 + nch], evz[:, c0:c0 + nch])
                    nc.vector.tensor_add(t1[:, :nch], t1[:, :nch], t2[:, :nch])
                    nc.scalar.activation(d_all[:, c0:c0 + nch], t1[:, :nch], AF.Sqrt)

                    # gaussian smearing -> feat_out[:, 0:128]
                    sm = sg.tile([128, SG_CH * NG], F32, tag="sm")
                    nc.vector.tensor_tensor(
                        out=sm[:, :nch * NG],
                        in0=_ap(d_all, d_all.offset + c0, [d_all.ap[0], [1, nch], [0, NG]]),
                        in1=_ap(offs_f, offs_f.offset, [offs_f.ap[0], [0, nch], [1, NG]]),
                        op=OP.subtract,
                    )
                    nc.vector.tensor_mul(sm[:, :nch * NG], sm[:, :nch * NG], sm[:, :nch * NG])
                    smear = sg.tile([128, SG_CH * NG], F32, tag="smear")
                    nc.scalar.activation(smear[:, :nch * NG], sm[:, :nch * NG], AF.Exp, scale=COEFF)
                    nc.sync.dma_start(
                        out=_ap(feat_out, e0 * 3 * NG,
                                [[3 * NG, 128], [3 * NG * 128, nch], [1, NG]]),
                        in_=_ap(smear, smear.offset, [smear.ap[0], [NG, nch], [1, NG]]),
                    )

                    # source/target embeddings via one-hot matmul -> feat_out[:, 128:384]
                    emb = sg.tile([128, SG_CH * 2 * C], F32, tag="emb")
                    for t in range(nch):
                        pe = ps.tile([128, 2 * C], F32, tag="pe")
                        for side, (pk, tab) in enumerate(((pk_src, srcw_sb), (pk_tgt, tgtw_sb))):
                            anb = _ap(pk, pk.offset + 4 * t + 3, [pk.ap[0], [0, 128]])
                            pT = ps.tile([128, 128], F32, tag=f"pT{side}")
                            nc.tensor.transpose(out=pT[:], in_=anb, identity=ident[:])
                            oh = sg.tile([MAX_ELEM, 128], F32, tag=f"oh{side}")
                            nc.vector.tensor_tensor(
                                out=oh[:], in0=pT[0:MAX_ELEM, :],
                                in1=_ap(iota_pf, iota_pf.offset, [[iota_pf.ap[0][0], MAX_ELEM], [0, 128]]),
                                op=OP.is_equal,
                            )
                            nc.tensor.matmul(
                                out=pe[:, side * C:(side + 1) * C],
                                lhsT=oh[:], rhs=(srcw_sb if side == 0 else tgtw_sb)[:],
                                start=True, stop=True,
                            )
                        nc.vector.tensor_copy(emb[:, 2 * C * t:2 * C * (t + 1)], pe[:])
                    nc.sync.dma_start(
                        out=_ap(feat_out, e0 * 3 * NG + NG,
                                [[3 * NG, 128], [3 * NG * 128, nch], [1, 2 * C]]),
                        in_=_ap(emb, emb.offset, [emb.ap[0], [2 * C, nch], [1, 2 * C]]),
                    )

            # ---- rotation matrices (SoA over [128, E_CH]) ----
            with tc.tile_pool(name="rot", bufs=1) as rp:
                rot_all = rp.tile([128, E_CH * 9], F32)
                HALF = (E_CH + 1) // 2
                for h0 in range(0, E_CH, HALF):
                    hw = min(HALF, E_CH - h0)

                    def rt(name):
                        return rp.tile([128, HALF], F32, name=f"{name}_{h0}", tag=name)[:, :hw]

                    sl = slice(h0, h0 + hw)
                    V = nc.vector
                    S = nc.scalar

                    nxx, nxy, nxz = rt("nxx"), rt("nxy"), rt("nxz")
                    rinv = rt("rinv")
                    V.reciprocal(rinv, d_all[:, sl])
                    V.tensor_mul(nxx, evx[:, sl], rinv)
                    V.tensor_mul(nxy, evy[:, sl], rinv)
                    V.tensor_mul(nxz, evz[:, sl], rinv)

                    e2x, e2y, e2z = rt("e2x"), rt("e2y"), rt("e2z")
                    for comp, dst in ((0, e2x), (1, e2y), (2, e2z)):
                        nc.sync.dma_start(
                            out=dst,
                            in_=_ap(rand, h0 * 128 * 3 + comp, [[3, 128], [384, hw]]),
                        )
                        V.tensor_scalar_add(dst, dst, -0.5)
                    s1, s2 = rt("s1"), rt("s2")
                    V.tensor_mul(s1, e2x, e2x)
                    V.tensor_mul(s2, e2y, e2y)
                    V.tensor_add(s1, s1, s2)
                    V.tensor_mul(s2, e2z, e2z)
                    V.tensor_add(s1, s1, s2)
                    S.activation(s1, s1, AF.Sqrt)
                    V.reciprocal(s2, s1)
                    V.tensor_mul(e2x, e2x, s2)
                    V.tensor_mul(e2y, e2y, s2)
                    V.tensor_mul(e2z, e2z, s2)

                    def adot(ax, ay, az, out, tmp):
                        V.tensor_mul(out, ax, nxx)
                        V.tensor_mul(tmp, ay, nxy)
                        V.tensor_add(out, out, tmp)
                        V.tensor_mul(tmp, az, nxz)
                        V.tensor_add(out, out, tmp)
                        S.activation(out, out, AF.Abs)

                    e2bx, e2by, e2bz = rt("e2bx"), rt("e2by"), rt("e2bz")
                    V.tensor_scalar_mul(e2bx, e2y, -1.0)
                    V.tensor_copy(e2by, e2x)
                    V.tensor_copy(e2bz, e2z)
                    e2cx, e2cy, e2cz = rt("e2cx"), rt("e2cy"), rt("e2cz")
                    V.tensor_copy(e2cx, e2x)
                    V.tensor_scalar_mul(e2cy, e2z, -1.0)
                    V.tensor_copy(e2cz, e2y)

                    vd, vdb, vdc, tmp = rt("vd"), rt("vdb"), rt("vdc"), rt("tmp")
                    adot(e2x, e2y, e2z, vd, tmp)
                    adot(e2bx, e2by, e2bz, vdb, tmp)
                    adot(e2cx, e2cy, e2cz, vdc, tmp)

                    mask = rp.tile([128, HALF], I32, name=f"mask_{h0}", tag="mask")[:, :hw]
                    V.tensor_tensor(out=mask, in0=vd, in1=vdb, op=OP.is_gt)
                    V.select(e2x, mask, e2bx, e2x)
                    V.select(e2y, mask, e2by, e2y)
                    V.select(e2z, mask, e2bz, e2z)

                    adot(e2x, e2y, e2z, vd, tmp)
                    V.tensor_tensor(out=mask, in0=vd, in1=vdc, op=OP.is_gt)
                    V.select(e2x, mask, e2cx, e2x)
                    V.select(e2y, mask, e2cy, e2y)
                    V.select(e2z, mask, e2cz, e2z)

                    def cross(ax, ay, az, bx, by, bz, ox, oy, oz, tmp):
                        V.tensor_mul(ox, ay, bz)
                        V.tensor_mul(tmp, az, by)
                        V.tensor_sub(ox, ox, tmp)
                        V.tensor_mul(oy, az, bx)
                        V.tensor_mul(tmp, ax, bz)
                        V.tensor_sub(oy, oy, tmp)
                        V.tensor_mul(oz, ax, by)
                        V.tensor_mul(tmp, ay, bx)
                        V.tensor_sub(oz, oz, tmp)

                    def normalize(ax, ay, az, s1, s2):
                        V.tensor_mul(s1, ax, ax)
                        V.tensor_mul(s2, ay, ay)
                        V.tensor_add(s1, s1, s2)
                        V.tensor_mul(s2, az, az)
                        V.tensor_add(s1, s1, s2)
                        S.activation(s1, s1, AF.Sqrt)
                        V.reciprocal(s2, s1)
                        V.tensor_mul(ax, ax, s2)
                        V.tensor_mul(ay, ay, s2)
                        V.tensor_mul(az, az, s2)

                    nzx, nzy, nzz = rt("nzx"), rt("nzy"), rt("nzz")
                    cross(nxx, nxy, nxz, e2x, e2y, e2z, nzx, nzy, nzz, tmp)
                    normalize(nzx, nzy, nzz, s1, s2)
                    nyx, nyy, nyz = rt("nyx"), rt("nyy"), rt("nyz")
                    cross(nxx, nxy, nxz, nzx, nzy, nzz, nyx, nyy, nyz, tmp)
                    normalize(nyx, nyy, nyz, s1, s2)

                    for j, src_t in enumerate((nzx, nzy, nzz, nxx, nxy, nxz, nyx, nyy, nyz)):
                        V.tensor_copy(
                            _ap(rot_all, rot_all.offset + h0 * 9 + j, [rot_all.ap[0], [9, hw]]),
                            src_t,
                        )
                nc.sync.dma_start(
                    out=_ap(rot_out, 0, [[9, 128], [9 * 128, E_CH], [1, 9]]),
                    in_=_ap(rot_all, rot_all.offset, [rot_all.ap[0], [9, E_CH], [1, 9]]),
                )

            # ---- node embeddings ----
            with tc.tile_pool(name="node", bufs=1) as npool:
                pw_i = npool.tile([128, 15], I32)
                nc.gpsimd.iota(pw_i[:], pattern=[[-1, 15]], base=14, channel_multiplier=0)
                one_i = npool.tile([128, 15], I32)
                nc.vector.memset(one_i[:], 1)
                nc.vector.tensor_tensor(out=pw_i[:], in0=one_i[:], in1=pw_i[:],
                                        op=mybir.AluOpType.logical_shift_left)
                pw_f = npool.tile([128, 15], F32)
                nc.vector.tensor_copy(pw_f[:], pw_i[:])

                an_t = npool.tile([128, NP_CH], I32)
                nc.sync.dma_start(out=an_t[:], in_=_ap(an_node, 0, [[1, 128], [128, NP_CH]]))

                xb_i = npool.tile([128, NP_CH * 15], I32)
                nc.sync.dma_start(
                    out=xb_i[:],
                    in_=_ap(xbits, 0, [[15, 128], [15 * 128, NP_CH], [1, 15]]),
                )
                xb_f = npool.tile([128, NP_CH * 15], F32)
                nc.vector.tensor_copy(xb_f[:], xb_i[:])
                prod = npool.tile([128, NP_CH * 15], F32)
                nc.vector.tensor_tensor(
                    out=prod[:],
                    in0=xb_f[:],
                    in1=_ap(pw_f, pw_f.offset, [pw_f.ap[0], [0, NP_CH], [1, 15]]),
                    op=mybir.AluOpType.mult,
                )
                xtra_f = npool.tile([128, NP_CH], F32)
                nc.vector.tensor_reduce(
                    out=xtra_f[:],
                    in_=_ap(prod, prod.offset, [prod.ap[0], [15, NP_CH], [1, 15]]),
                    axis=mybir.AxisListType.X,
                    op=mybir.AluOpType.add,
                )
                nc.vector.tensor_scalar_min(xtra_f[:], xtra_f[:], float(EMB2 - 1))
                xtra_i = npool.tile([128, NP_CH], I32)
                nc.vector.tensor_copy(xtra_i[:], xtra_f[:])

                g1 = npool.tile([128, NP_CH * C], F32)
                g2 = npool.tile([128, NP_CH * C], F32)
                for t in range(NP_CH):
                    nc.gpsimd.indirect_dma_start(
                        out=g1[:, C * t:C * (t + 1)], out_offset=None, in_=sphere_w[:],
                        in_offset=IndirectOffsetOnAxis(ap=an_t[:, t:t + 1], axis=0),
                    )
                    nc.gpsimd.indirect_dma_start(
                        out=g2[:, C * t:C * (t + 1)], out_offset=None, in_=sphere2_w[:],
                        in_offset=IndirectOffsetOnAxis(ap=xtra_i[:, t:t + 1], axis=0),
                    )
                nc.vector.tensor_add(g1[:], g1[:], g2[:])
                nc.sync.dma_start(
                    out=_ap(node_out, 0,
                            [[NUM_COEF * C, 128], [NUM_COEF * C * 128, NP_CH], [1, C]]),
                    in_=_ap(g1, g1.offset, [g1.ap[0], [C, NP_CH], [1, C]]),
                )
                nc.sync.dma_start(
                    out=_ap(node_out, C,
                            [[NUM_COEF * C, 128], [NUM_COEF * C * 128, NP_CH],
                             [1, (NUM_COEF - 1) * C]]),
                    in_=_ap(zero_t, zero_t.offset,
                            [zero_t.ap[0], [0, NP_CH], [1, (NUM_COEF - 1) * C]]),
                )

    nc.compile()
    return nc


_NC = None


def kernel(**inputs):
    global _NC, LAST_RESULT
    if _NC is None:
        _NC = build_module()
    nc = _NC

    pos = np.ascontiguousarray(np.asarray(inputs["pos"], dtype=np.float32))
    sphere_w = np.ascontiguousarray(np.asarray(inputs["sphere_w"], dtype=np.float32))
    sphere2_w = np.ascontiguousarray(np.asarray(inputs["sphere2_w"], dtype=np.float32))
    source_w = np.ascontiguousarray(np.asarray(inputs["source_w"], dtype=np.float32))
    target_w = np.ascontiguousarray(np.asarray(inputs["target_w"], dtype=np.float32))
    rand_vec = np.asarray(inputs["rand_vec"], dtype=np.float32)
    an = np.asarray(inputs["atomic_numbers"]).astype(np.int32)
    x_bits = np.asarray(inputs["x_bits"]).astype(np.int32)
    edge_index = np.asarray(inputs["edge_index"]).astype(np.int32)

    in_maps = []
    for c in range(N_CORES):
        e0 = c * E_PER
        src = np.zeros(E_PAD, np.int32)
        tgt = np.zeros(E_PAD, np.int32)
        src[:E_PER] = edge_index[0, e0:e0 + E_PER]
        tgt[:E_PER] = edge_index[1, e0:e0 + E_PER]
        tgt[E_PER:] = 1  # distinct from src pad (0) so padded edges stay finite
        rnd = np.full((E_PAD, 3), 0.25, np.float32)
        rnd[:E_PER] = rand_vec[e0:e0 + E_PER]
        n0 = c * NP_PER
        an_n = np.zeros(NP_PAD, np.int32)
        an_n[:NP_PER] = an[n0:n0 + NP_PER]
        xb = np.zeros((NP_PAD, 15), np.int32)
        xb[:NP_PER] = x_bits[n0:n0 + NP_PER]
        in_maps.append({
            "pos": pos, "sphere_w": sphere_w, "sphere2_w": sphere2_w,
            "source_w": source_w, "target_w": target_w, "an_full": an,
            "src_idx": np.ascontiguousarray(src), "tgt_idx": np.ascontiguousarray(tgt),
            "rand": np.ascontiguousarray(rnd), "an_node": an_n,
            "xbits": np.ascontiguousarray(xb),
        })

    res = run_bass_kernel_spmd(nc, in_maps, core_ids=list(range(N_CORES)), trace=TRACE)
    LAST_RESULT = res

    node_emb = np.concatenate([res.results[c]["node_out"][:NP_PER] for c in range(N_CORES)], axis=0)
    edge_feat = np.concatenate([res.results[c]["feat_out"][:E_PER] for c in range(N_CORES)], axis=0)
    rot = np.concatenate([res.results[c]["rot_out"][:E_PER] for c in range(N_CORES)], axis=0)
    return node_emb, edge_feat, rot.reshape(N_EDGES, 3, 3)


# revision 9
# speedup vs baseline: 1.0088x; 1.0088x over previous
"""Trainium2 Bass kernel for nn_EquivariantEmbedding (gnn_message_passing).

Sharding: edge-parallel across 8 cores (58500 edges each, padded to 58624);
node embedding data-parallel (2500 nodes/core, padded to 2560). Small
embedding tables are replicated. All compute (gathers, rotation matrices,
gaussian smearing, source/target one-hot embeddings, bit-packing) happens
on device.

Per-edge node data (pos + atomic number, packed 16B rows) is gathered with
SWDGE indirect DMAs, 128 rows per op. Source/target embeddings are computed
on the tensor engine as one-hot(atomic number) @ table matmuls — no 512B-row
gathers needed. Rotation matrices are computed in a structure-of-arrays
layout with full-width vector ops.
"""
import sys

for _p in ("/opt/trn_rl_repo",):
    if _p not in sys.path:
        sys.path.append(_p)

import numpy as np

import concourse.bacc as bacc
import concourse.bass as bass
import concourse.tile as tile
from concourse import mybir
from concourse.bass import IndirectOffsetOnAxis
from concourse.bass_utils import run_bass_kernel_spmd
from concourse.masks import make_identity

F32 = mybir.dt.float32
BF16 = mybir.dt.bfloat16
I32 = mybir.dt.int32

N_CORES = 8
N_NODES = 20000
N_EDGES = 468000
C = 128
NG = 128
NUM_COEF = 16
MAX_ELEM = 90
EMB2 = 32767
CUTOFF = 5.0
BWS = 20.0

E_PER = N_EDGES // N_CORES            # 58500
E_CH = (E_PER + 127) // 128           # 458 chunks of 128 edges
E_PAD = E_CH * 128                    # 58624

NP_PER = N_NODES // N_CORES           # 2500
NP_CH = (NP_PER + 127) // 128         # 20
NP_PAD = NP_CH * 128                  # 2560

SG_CH = 16                            # chunks per supergroup (2048 edges)

_offs = np.linspace(0.0, CUTOFF, NG, dtype=np.float32)
OFF_STEP = float(_offs[1] - _offs[0])
COEFF = float(-0.5 / (BWS * (_offs[1] - _offs[0])) ** 2)

TRACE = False  # test.py sets this for profiled runs
LAST_RESULT = None


def _ap(t, offset, dims):
    return bass.AP(tensor=t.tensor if hasattr(t, "tensor") else t, offset=offset,
                   ap=[list(d) for d in dims])


def build_module():
    nc = bacc.Bacc("TRN2", target_bir_lowering=False, debug=False, num_devices=N_CORES)

    pos = nc.dram_tensor("pos", [N_NODES, 3], F32, kind="ExternalInput")
    sphere_w = nc.dram_tensor("sphere_w", [MAX_ELEM, C], F32, kind="ExternalInput")
    sphere2_w = nc.dram_tensor("sphere2_w", [EMB2, C], F32, kind="ExternalInput")
    source_w = nc.dram_tensor("source_w", [MAX_ELEM, NG], F32, kind="ExternalInput")
    target_w = nc.dram_tensor("target_w", [MAX_ELEM, NG], F32, kind="ExternalInput")
    an_full = nc.dram_tensor("an_full", [N_NODES], I32, kind="ExternalInput")
    src_idx = nc.dram_tensor("src_idx", [E_PAD], I32, kind="ExternalInput")
    tgt_idx = nc.dram_tensor("tgt_idx", [E_PAD], I32, kind="ExternalInput")
    rand = nc.dram_tensor("rand", [E_PAD, 3], F32, kind="ExternalInput")
    an_node = nc.dram_tensor("an_node", [NP_PAD], I32, kind="ExternalInput")
    xbits = nc.dram_tensor("xbits", [NP_PAD, 15], I32, kind="ExternalInput")

    node_out = nc.dram_tensor("node_out", [NP_PAD, NUM_COEF, C], F32, kind="ExternalOutput")
    feat_out = nc.dram_tensor("feat_out", [E_PAD, 3 * NG], F32, kind="ExternalOutput")
    rot_out = nc.dram_tensor("rot_out", [E_PAD, 9], F32, kind="ExternalOutput")

    AF = mybir.ActivationFunctionType
    OP = mybir.AluOpType

    with tile.TileContext(nc) as tc:
        import contextlib
        with contextlib.ExitStack() as ctx:
            consts = ctx.enter_context(tc.tile_pool(name="consts", bufs=1))
            dram = ctx.enter_context(tc.tile_pool(name="dram", bufs=1, space="DRAM"))

            # ---- constants ----
            offs_i = consts.tile([128, NG], I32)
            nc.gpsimd.iota(offs_i[:], pattern=[[1, NG]], base=0, channel_multiplier=0)
            offs_f = consts.tile([128, NG], F32)
            nc.vector.tensor_copy(offs_f[:], offs_i[:])
            nc.vector.tensor_scalar_mul(offs_f[:], offs_f[:], OFF_STEP)

            zero_t = consts.tile([128, (NUM_COEF - 1) * C], F32)
            nc.vector.memset(zero_t[:], 0.0)

            ident = consts.tile([128, 128], BF16)
            make_identity(nc, ident[:])

            # iota down partitions (value = partition index) as f32, for one-hots
            iota_p = consts.tile([128, 1], I32)
            nc.gpsimd.iota(iota_p[:], pattern=[[0, 1]], base=0, channel_multiplier=1)
            iota_pf = consts.tile([128, 1], F32)
            nc.vector.tensor_copy(iota_pf[:], iota_p[:])
            iota_pb = consts.tile([128, 1], BF16)
            nc.vector.tensor_copy(iota_pb[:], iota_p[:])

            # embedding tables resident in SBUF, split into bf16 hi+lo
            # (hi + lo == f32 value to ~2^-18 relative; matmul accumulates in f32)
            def split_table(dram_t, name):
                f = consts.tile([MAX_ELEM, C], F32, name=name + "_f")
                nc.sync.dma_start(out=f[:], in_=dram_t[:])
                hi = consts.tile([MAX_ELEM, C], BF16, name=name + "_hi")
                nc.vector.tensor_copy(hi[:], f[:])
                lo_f = consts.tile([MAX_ELEM, C], F32, name=name + "_lof")
                nc.vector.tensor_sub(lo_f[:], f[:], hi[:])
                lo = consts.tile([MAX_ELEM, C], BF16, name=name + "_lo")
                nc.vector.tensor_copy(lo[:], lo_f[:])
                return hi, lo

            srcw_hi, srcw_lo = split_table(source_w, "srcw")
            tgtw_hi, tgtw_lo = split_table(target_w, "tgtw")

            # ---- persistent edge-index tiles: idx32[p, ch] = idx[128*ch + p] ----
            s_i32 = consts.tile([128, E_CH], I32, name="s_i32")
            t_i32 = consts.tile([128, E_CH], I32, name="t_i32")
            nc.sync.dma_start(out=s_i32[:], in_=_ap(src_idx, 0, [[1, 128], [128, E_CH]]))
            nc.sync.dma_start(out=t_i32[:], in_=_ap(tgt_idx, 0, [[1, 128], [128, E_CH]]))

            # ---- packed node table [N_NODES, 4]: (x, y, z, float(an)) ----
            packed = dram.tile([N_NODES, 4], F32)
            nc.sync.dma_start(
                out=_ap(packed, packed.offset, [[4, N_NODES], [1, 3]]),
                in_=pos[:],
            )
            NACH = (N_NODES + 127) // 128  # 157
            an_sb = consts.tile([128, NACH], I32, name="an_sb")
            nc.vector.memset(an_sb[:], 0)
            nc.sync.dma_start(
                out=an_sb[:, :NACH - 1],
                in_=_ap(an_full, 0, [[1, 128], [128, NACH - 1]]),
            )
            nc.sync.dma_start(  # tail chunk (32 nodes)
                out=an_sb[0:N_NODES - 128 * (NACH - 1), NACH - 1:NACH],
                in_=_ap(an_full, 128 * (NACH - 1), [[1, N_NODES - 128 * (NACH - 1)], [1, 1]]),
            )
            an_f = consts.tile([128, NACH], F32, name="an_f")
            nc.vector.tensor_copy(an_f[:], an_sb[:])
            nc.sync.dma_start(
                out=_ap(packed, packed.offset + 3, [[4, 128], [512, NACH - 1], [1, 1]]),
                in_=an_f[:, :NACH - 1],
            )
            nc.sync.dma_start(
                out=_ap(packed, packed.offset + 3 + 512 * (NACH - 1),
                        [[4, N_NODES - 128 * (NACH - 1)], [1, 1]]),
                in_=an_f[0:N_NODES - 128 * (NACH - 1), NACH - 1:NACH],
            )

            # ---- SoA tiles for the rotation-matrix stage ----
            evx = consts.tile([128, E_CH], F32, name="evx")
            evy = consts.tile([128, E_CH], F32, name="evy")
            evz = consts.tile([128, E_CH], F32, name="evz")
            d_all = consts.tile([128, E_CH], F32, name="d_all")

            sg_bounds = []
            ch0 = 0
            while ch0 < E_CH:
                sg_bounds.append((ch0, min(SG_CH, E_CH - ch0)))
                ch0 += SG_CH

            with tc.tile_pool(name="sg", bufs=3) as sg, \
                 tc.tile_pool(name="ps", bufs=2, space="PSUM") as ps:
                for (c0, nch) in sg_bounds:
                    e0 = c0 * 128
                    # gather packed rows for src/tgt, one [128,1]-offset op per chunk
                    pk_src = sg.tile([128, SG_CH * 4], F32, tag="psrc")
                    pk_tgt = sg.tile([128, SG_CH * 4], F32, tag="ptgt")
                    for t in range(nch):
                        nc.gpsimd.indirect_dma_start(
                            out=pk_src[:, 4 * t:4 * t + 4], out_offset=None,
                            in_=packed[:],
                            in_offset=IndirectOffsetOnAxis(ap=s_i32[:, c0 + t:c0 + t + 1], axis=0),
                        )
                        nc.gpsimd.indirect_dma_start(
                            out=pk_tgt[:, 4 * t:4 * t + 4], out_offset=None,
                            in_=packed[:],
                            in_offset=IndirectOffsetOnAxis(ap=t_i32[:, c0 + t:c0 + t + 1], axis=0),
                        )
                    # ev components into SoA tiles
                    for comp, dst in ((0, evx), (1, evy), (2, evz)):
                        nc.vector.tensor_tensor(
                            out=dst[:, c0:c0 + nch],
                            in0=_ap(pk_src, pk_src.offset + comp, [pk_src.ap[0], [4, nch]]),
                            in1=_ap(pk_tgt, pk_tgt.offset + comp, [pk_tgt.ap[0], [4, nch]]),
                            op=OP.subtract,
                        )
                    # d = sqrt(evx^2 + evy^2 + evz^2)
                    t1 = sg.tile([128, SG_CH], F32, tag="t1")
                    t2 = sg.tile([128, SG_CH], F32, tag="t2")
                    nc.vector.tensor_mul(t1[:, :nch], evx[:, c0:c0 + nch], evx[:, c0:c0 + nch])
                    nc.vector.tensor_mul(t2[:, :nch], evy[:, c0:c0 + nch], evy[:, c0:c0 + nch])
                    nc.vector.tensor_add(t1[:, :nch], t1[:, :nch], t2[:, :nch])
                    nc.vector.tensor_mul(t2[:, :nch], evz[:, c0:c0 + nch], evz[:, c0:c0 + nch])
                    nc.vector.tensor_add(t1[:, :nch], t1[:, :nch], t2[:, :nch])
                    nc.scalar.activation(d_all[:, c0:c0 + nch], t1[:, :nch], AF.Sqrt)

                    # gaussian smearing -> feat_out[:, 0:128]
                    sm = sg.tile([128, SG_CH * NG], F32, tag="sm")
                    nc.vector.tensor_tensor(
                        out=sm[:, :nch * NG],
                        in0=_ap(d_all, d_all.offset + c0, [d_all.ap[0], [1, nch], [0, NG]]),
                        in1=_ap(offs_f, offs_f.offset, [offs_f.ap[0], [0, nch], [1, NG]]),
                        op=OP.subtract,
                    )
                    nc.vector.tensor_mul(sm[:, :nch * NG], sm[:, :nch * NG], sm[:, :nch * NG])
                    smear = sg.tile([128, SG_CH * NG], F32, tag="smear")
                    nc.scalar.activation(smear[:, :nch * NG], sm[:, :nch * NG], AF.Exp, scale=COEFF)
                    nc.sync.dma_start(
                        out=_ap(feat_out, e0 * 3 * NG,
                                [[3 * NG, 128], [3 * NG * 128, nch], [1, NG]]),
                        in_=_ap(smear, smear.offset, [smear.ap[0], [NG, nch], [1, NG]]),
                    )

                    # source/target embeddings via bf16 one-hot matmul -> feat_out[:, 128:384]
                    an_bf = sg.tile([128, SG_CH * 2], BF16, tag="anbf")
                    nc.vector.tensor_copy(
                        an_bf[:, 0:nch],
                        _ap(pk_src, pk_src.offset + 3, [pk_src.ap[0], [4, nch]]),
                    )
                    nc.vector.tensor_copy(
                        an_bf[:, SG_CH:SG_CH + nch],
                        _ap(pk_tgt, pk_tgt.offset + 3, [pk_tgt.ap[0], [4, nch]]),
                    )
                    emb = sg.tile([128, SG_CH * 2 * C], F32, tag="emb")
                    for t in range(nch):
                        pe = ps.tile([128, 2 * C], F32, tag="pe")
                        for side, (hi, lo) in enumerate(((srcw_hi, srcw_lo), (tgtw_hi, tgtw_lo))):
                            col = t if side == 0 else SG_CH + t
                            anb = _ap(an_bf, an_bf.offset + col, [an_bf.ap[0], [0, 128]])
                            pT = ps.tile([128, 128], BF16, tag=f"pT{side}")
                            nc.tensor.transpose(out=pT[:], in_=anb, identity=ident[:])
                            oh = sg.tile([MAX_ELEM, 128], BF16, tag=f"oh{side}")
                            nc.vector.tensor_tensor(
                                out=oh[:], in0=pT[0:MAX_ELEM, :],
                                in1=_ap(iota_pb, iota_pb.offset, [[iota_pb.ap[0][0], MAX_ELEM], [0, 128]]),
                                op=OP.is_equal,
                            )
                            nc.tensor.matmul(
                                out=pe[:, side * C:(side + 1) * C],
                                lhsT=oh[:], rhs=hi[:], start=True, stop=False,
                            )
                            nc.tensor.matmul(
                                out=pe[:, side * C:(side + 1) * C],
                                lhsT=oh[:], rhs=lo[:], start=False, stop=True,
                            )
                        nc.vector.tensor_copy(emb[:, 2 * C * t:2 * C * (t + 1)], pe[:])
                    nc.sync.dma_start(
                        out=_ap(feat_out, e0 * 3 * NG + NG,
                                [[3 * NG, 128], [3 * NG * 128, nch], [1, 2 * C]]),
                        in_=_ap(emb, emb.offset, [emb.ap[0], [2 * C, nch], [1, 2 * C]]),
                    )

            # ---- rotation matrices (SoA over [128, E_CH]) ----
            with tc.tile_pool(name="rot", bufs=1) as rp:
                rot_all = rp.tile([128, E_CH * 9], F32)
                HALF = (E_CH + 1) // 2
                for h0 in range(0, E_CH, HALF):
                    hw = min(HALF, E_CH - h0)

                    def rt(name):
                        return rp.tile([128, HALF], F32, name=f"{name}_{h0}", tag=name)[:, :hw]

                    sl = slice(h0, h0 + hw)
                    V = nc.vector
                    S = nc.scalar

                    nxx, nxy, nxz = rt("nxx"), rt("nxy"), rt("nxz")
                    rinv = rt("rinv")
                    V.reciprocal(rinv, d_all[:, sl])
                    V.tensor_mul(nxx, evx[:, sl], rinv)
                    V.tensor_mul(nxy, evy[:, sl], rinv)
                    V.tensor_mul(nxz, evz[:, sl], rinv)

                    e2x, e2y, e2z = rt("e2x"), rt("e2y"), rt("e2z")
                    for comp, dst in ((0, e2x), (1, e2y), (2, e2z)):
                        nc.sync.dma_start(
                            out=dst,
                            in_=_ap(rand, h0 * 128 * 3 + comp, [[3, 128], [384, hw]]),
                        )
                        V.tensor_scalar_add(dst, dst, -0.5)
                    s1, s2 = rt("s1"), rt("s2")
                    V.tensor_mul(s1, e2x, e2x)
                    V.tensor_mul(s2, e2y, e2y)
                    V.tensor_add(s1, s1, s2)
                    V.tensor_mul(s2, e2z, e2z)
                    V.tensor_add(s1, s1, s2)
                    S.activation(s1, s1, AF.Sqrt)
                    V.reciprocal(s2, s1)
                    V.tensor_mul(e2x, e2x, s2)
                    V.tensor_mul(e2y, e2y, s2)
                    V.tensor_mul(e2z, e2z, s2)

                    def adot(ax, ay, az, out, tmp):
                        V.tensor_mul(out, ax, nxx)
                        V.tensor_mul(tmp, ay, nxy)
                        V.tensor_add(out, out, tmp)
                        V.tensor_mul(tmp, az, nxz)
                        V.tensor_add(out, out, tmp)
                        S.activation(out, out, AF.Abs)

                    e2bx, e2by, e2bz = rt("e2bx"), rt("e2by"), rt("e2bz")
                    V.tensor_scalar_mul(e2bx, e2y, -1.0)
                    V.tensor_copy(e2by, e2x)
                    V.tensor_copy(e2bz, e2z)
                    e2cx, e2cy, e2cz = rt("e2cx"), rt("e2cy"), rt("e2cz")
                    V.tensor_copy(e2cx, e2x)
                    V.tensor_scalar_mul(e2cy, e2z, -1.0)
                    V.tensor_copy(e2cz, e2y)

                    vd, vdb, vdc, tmp = rt("vd"), rt("vdb"), rt("vdc"), rt("tmp")
                    adot(e2x, e2y, e2z, vd, tmp)
                    adot(e2bx, e2by, e2bz, vdb, tmp)
                    adot(e2cx, e2cy, e2cz, vdc, tmp)

                    mask = rp.tile([128, HALF], I32, name=f"mask_{h0}", tag="mask")[:, :hw]
                    V.tensor_tensor(out=mask, in0=vd, in1=vdb, op=OP.is_gt)
                    V.select(e2x, mask, e2bx, e2x)
                    V.select(e2y, mask, e2by, e2y)
                    V.select(e2z, mask, e2bz, e2z)

                    adot(e2x, e2y, e2z, vd, tmp)
                    V.tensor_tensor(out=mask, in0=vd, in1=vdc, op=OP.is_gt)
                    V.select(e2x, mask, e2cx, e2x)
                    V.select(e2y, mask, e2cy, e2y)
                    V.select(e2z, mask, e2cz, e2z)

                    def cross(ax, ay, az, bx, by, bz, ox, oy, oz, tmp):
                        V.tensor_mul(ox, ay, bz)
                        V.tensor_mul(tmp, az, by)
                        V.tensor_sub(ox, ox, tmp)
                        V.tensor_mul(oy, az, bx)
                        V.tensor_mul(tmp, ax, bz)
                        V.tensor_sub(oy, oy, tmp)
                        V.tensor_mul(oz, ax, by)
                        V.tensor_mul(tmp, ay, bx)
                        V.tensor_sub(oz, oz, tmp)

                    def normalize(ax, ay, az, s1, s2):
                        V.tensor_mul(s1, ax, ax)
                        V.tensor_mul(s2, ay, ay)
                        V.tensor_add(s1, s1, s2)
                        V.tensor_mul(s2, az, az)
                        V.tensor_add(s1, s1, s2)
                        S.activation(s1, s1, AF.Sqrt)
                        V.reciprocal(s2, s1)
                        V.tensor_mul(ax, ax, s2)
                        V.tensor_mul(ay, ay, s2)
                        V.tensor_mul(az, az, s2)

                    nzx, nzy, nzz = rt("nzx"), rt("nzy"), rt("nzz")
                    cross(nxx, nxy, nxz, e2x, e2y, e2z, nzx, nzy, nzz, tmp)
                    normalize(nzx, nzy, nzz, s1, s2)
                    nyx, nyy, nyz = rt("nyx"), rt("nyy"), rt("nyz")
                    cross(nxx, nxy, nxz, nzx, nzy, nzz, nyx, nyy, nyz, tmp)
                    normalize(nyx, nyy, nyz, s1, s2)

                    for j, src_t in enumerate((nzx, nzy, nzz, nxx, nxy, nxz, nyx, nyy, nyz)):
                        V.tensor_copy(
                            _ap(rot_all, rot_all.offset + h0 * 9 + j, [rot_all.ap[0], [9, hw]]),
                            src_t,
                        )
                nc.sync.dma_start(
                    out=_ap(rot_out, 0, [[9, 128], [9 * 128, E_CH], [1, 9]]),
                    in_=_ap(rot_all, rot_all.offset, [rot_all.ap[0], [9, E_CH], [1, 9]]),
                )

            # ---- node embeddings ----
            with tc.tile_pool(name="node", bufs=1) as npool:
                pw_i = npool.tile([128, 15], I32)
                nc.gpsimd.iota(pw_i[:], pattern=[[-1, 15]], base=14, channel_multiplier=0)
                one_i = npool.tile([128, 15], I32)
                nc.vector.memset(one_i[:], 1)
                nc.vector.tensor_tensor(out=pw_i[:], in0=one_i[:], in1=pw_i[:],
                                        op=mybir.AluOpType.logical_shift_left)
                pw_f = npool.tile([128, 15], F32)
                nc.vector.tensor_copy(pw_f[:], pw_i[:])

                an_t = npool.tile([128, NP_CH], I32)
                nc.sync.dma_start(out=an_t[:], in_=_ap(an_node, 0, [[1, 128], [128, NP_CH]]))

                xb_i = npool.tile([128, NP_CH * 15], I32)
                nc.sync.dma_start(
                    out=xb_i[:],
                    in_=_ap(xbits, 0, [[15, 128], [15 * 128, NP_CH], [1, 15]]),
                )
                xb_f = npool.tile([128, NP_CH * 15], F32)
                nc.vector.tensor_copy(xb_f[:], xb_i[:])
                prod = npool.tile([128, NP_CH * 15], F32)
                nc.vector.tensor_tensor(
                    out=prod[:],
                    in0=xb_f[:],
                    in1=_ap(pw_f, pw_f.offset, [pw_f.ap[0], [0, NP_CH], [1, 15]]),
                    op=mybir.AluOpType.mult,
                )
                xtra_f = npool.tile([128, NP_CH], F32)
                nc.vector.tensor_reduce(
                    out=xtra_f[:],
                    in_=_ap(prod, prod.offset, [prod.ap[0], [15, NP_CH], [1, 15]]),
                    axis=mybir.AxisListType.X,
                    op=mybir.AluOpType.add,
                )
                nc.vector.tensor_scalar_min(xtra_f[:], xtra_f[:], float(EMB2 - 1))
                xtra_i = npool.tile([128, NP_CH], I32)
                nc.vector.tensor_copy(xtra_i[:], xtra_f[:])

                g1 = npool.tile([128, NP_CH * C], F32)
                g2 = npool.tile([128, NP_CH * C], F32)
                for t in range(NP_CH):
                    nc.gpsimd.indirect_dma_start(
                        out=g1[:, C * t:C * (t + 1)], out_offset=None, in_=sphere_w[:],
                        in_offset=IndirectOffsetOnAxis(ap=an_t[:, t:t + 1], axis=0),
                    )
                    nc.gpsimd.indirect_dma_start(
                        out=g2[:, C * t:C * (t + 1)], out_offset=None, in_=sphere2_w[:],
                        in_offset=IndirectOffsetOnAxis(ap=xtra_i[:, t:t + 1], axis=0),
                    )
                nc.vector.tensor_add(g1[:], g1[:], g2[:])
                nc.sync.dma_start(
                    out=_ap(node_out, 0,
                            [[NUM_COEF * C, 128], [NUM_COEF * C * 128, NP_CH], [1, C]]),
                    in_=_ap(g1, g1.offset, [g1.ap[0], [C, NP_CH], [1, C]]),
                )
                nc.sync.dma_start(
                    out=_ap(node_out, C,
                            [[NUM_COEF * C, 128], [NUM_COEF * C * 128, NP_CH],
                             [1, (NUM_COEF - 1) * C]]),
                    in_=_ap(zero_t, zero_t.offset,
                            [zero_t.ap[0], [0, NP_CH], [1, (NUM_COEF - 1) * C]]),
                )

    nc.compile()
    return nc


_NC = None


def kernel(**inputs):
    global _NC, LAST_RESULT
    if _NC is None:
        _NC = build_module()
    nc = _NC

    pos = np.ascontiguousarray(np.asarray(inputs["pos"], dtype=np.float32))
    sphere_w = np.ascontiguousarray(np.asarray(inputs["sphere_w"], dtype=np.float32))
    sphere2_w = np.ascontiguousarray(np.asarray(inputs["sphere2_w"], dtype=np.float32))
    source_w = np.ascontiguousarray(np.asarray(inputs["source_w"], dtype=np.float32))
    target_w = np.ascontiguousarray(np.asarray(inputs["target_w"], dtype=np.float32))
    rand_vec = np.asarray(inputs["rand_vec"], dtype=np.float32)
    an = np.asarray(inputs["atomic_numbers"]).astype(np.int32)
    x_bits = np.asarray(inputs["x_bits"]).astype(np.int32)
    edge_index = np.asarray(inputs["edge_index"]).astype(np.int32)

    in_maps = []
    for c in range(N_CORES):
        e0 = c * E_PER
        src = np.zeros(E_PAD, np.int32)
        tgt = np.zeros(E_PAD, np.int32)
        src[:E_PER] = edge_index[0, e0:e0 + E_PER]
        tgt[:E_PER] = edge_index[1, e0:e0 + E_PER]
        tgt[E_PER:] = 1  # distinct from src pad (0) so padded edges stay finite
        rnd = np.full((E_PAD, 3), 0.25, np.float32)
        rnd[:E_PER] = rand_vec[e0:e0 + E_PER]
        n0 = c * NP_PER
        an_n = np.zeros(NP_PAD, np.int32)
        an_n[:NP_PER] = an[n0:n0 + NP_PER]
        xb = np.zeros((NP_PAD, 15), np.int32)
        xb[:NP_PER] = x_bits[n0:n0 + NP_PER]
        in_maps.append({
            "pos": pos, "sphere_w": sphere_w, "sphere2_w": sphere2_w,
            "source_w": source_w, "target_w": target_w, "an_full": an,
            "src_idx": np.ascontiguousarray(src), "tgt_idx": np.ascontiguousarray(tgt),
            "rand": np.ascontiguousarray(rnd), "an_node": an_n,
            "xbits": np.ascontiguousarray(xb),
        })

    res = run_bass_kernel_spmd(nc, in_maps, core_ids=list(range(N_CORES)), trace=TRACE)
    LAST_RESULT = res

    node_emb = np.concatenate([res.results[c]["node_out"][:NP_PER] for c in range(N_CORES)], axis=0)
    edge_feat = np.concatenate([res.results[c]["feat_out"][:E_PER] for c in range(N_CORES)], axis=0)
    rot = np.concatenate([res.results[c]["rot_out"][:E_PER] for c in range(N_CORES)], axis=0)
    return node_emb, edge_feat, rot.reshape(N_EDGES, 3, 3)


# revision 10
# speedup vs baseline: 1.0197x; 1.0108x over previous
"""Trainium2 Bass kernel for nn_EquivariantEmbedding (gnn_message_passing).

Sharding: edge-parallel across 8 cores (58500 edges each, padded to 58624);
node embedding data-parallel (2500 nodes/core, padded to 2560). Small
embedding tables are replicated. All compute (gathers, rotation matrices,
gaussian smearing, source/target one-hot embeddings, bit-packing) happens
on device.

Per-edge node data (pos + atomic number, packed 16B rows) is gathered with
SWDGE indirect DMAs, 128 rows per op. Source/target embeddings are computed
on the tensor engine as one-hot(atomic number) @ table matmuls — no 512B-row
gathers needed. Rotation matrices are computed in a structure-of-arrays
layout with full-width vector ops.
"""
import sys

for _p in ("/opt/trn_rl_repo",):
    if _p not in sys.path:
        sys.path.append(_p)

import numpy as np

import concourse.bacc as bacc
import concourse.bass as bass
import concourse.tile as tile
from concourse import mybir
from concourse.bass import IndirectOffsetOnAxis
from concourse.bass_utils import run_bass_kernel_spmd
from concourse.masks import make_identity

F32 = mybir.dt.float32
BF16 = mybir.dt.bfloat16
I32 = mybir.dt.int32

N_CORES = 8
N_NODES = 20000
N_EDGES = 468000
C = 128
NG = 128
NUM_COEF = 16
MAX_ELEM = 90
EMB2 = 32767
CUTOFF = 5.0
BWS = 20.0

E_PER = N_EDGES // N_CORES            # 58500
E_CH = (E_PER + 127) // 128           # 458 chunks of 128 edges
E_PAD = E_CH * 128                    # 58624

NP_PER = N_NODES // N_CORES           # 2500
NP_CH = (NP_PER + 127) // 128         # 20
NP_PAD = NP_CH * 128                  # 2560

SG_CH = 16                            # chunks per supergroup (2048 edges)

_offs = np.linspace(0.0, CUTOFF, NG, dtype=np.float32)
OFF_STEP = float(_offs[1] - _offs[0])
COEFF = float(-0.5 / (BWS * (_offs[1] - _offs[0])) ** 2)

TRACE = False  # test.py sets this for profiled runs
LAST_RESULT = None


def _ap(t, offset, dims):
    return bass.AP(tensor=t.tensor if hasattr(t, "tensor") else t, offset=offset,
                   ap=[list(d) for d in dims])


def build_module():
    nc = bacc.Bacc("TRN2", target_bir_lowering=False, debug=False, num_devices=N_CORES)

    pos = nc.dram_tensor("pos", [N_NODES, 3], F32, kind="ExternalInput")
    sphere_w = nc.dram_tensor("sphere_w", [MAX_ELEM, C], F32, kind="ExternalInput")
    sphere2_w = nc.dram_tensor("sphere2_w", [EMB2, C], F32, kind="ExternalInput")
    source_w = nc.dram_tensor("source_w", [MAX_ELEM, NG], F32, kind="ExternalInput")
    target_w = nc.dram_tensor("target_w", [MAX_ELEM, NG], F32, kind="ExternalInput")
    an_full = nc.dram_tensor("an_full", [N_NODES], I32, kind="ExternalInput")
    src_idx = nc.dram_tensor("src_idx", [E_PAD], I32, kind="ExternalInput")
    tgt_idx = nc.dram_tensor("tgt_idx", [E_PAD], I32, kind="ExternalInput")
    rand = nc.dram_tensor("rand", [E_PAD, 3], F32, kind="ExternalInput")
    an_node = nc.dram_tensor("an_node", [NP_PAD], I32, kind="ExternalInput")
    xbits = nc.dram_tensor("xbits", [NP_PAD, 15], I32, kind="ExternalInput")

    node_out = nc.dram_tensor("node_out", [NP_PAD, NUM_COEF, C], F32, kind="ExternalOutput")
    feat_out = nc.dram_tensor("feat_out", [E_PAD, 3 * NG], F32, kind="ExternalOutput")
    rot_out = nc.dram_tensor("rot_out", [E_PAD, 9], F32, kind="ExternalOutput")

    AF = mybir.ActivationFunctionType
    OP = mybir.AluOpType

    with tile.TileContext(nc) as tc:
        import contextlib
        with contextlib.ExitStack() as ctx:
            consts = ctx.enter_context(tc.tile_pool(name="consts", bufs=1))
            dram = ctx.enter_context(tc.tile_pool(name="dram", bufs=1, space="DRAM"))

            # ---- constants ----
            offs_i = consts.tile([128, NG], I32)
            nc.gpsimd.iota(offs_i[:], pattern=[[1, NG]], base=0, channel_multiplier=0)
            offs_f = consts.tile([128, NG], F32)
            nc.vector.tensor_copy(offs_f[:], offs_i[:])
            nc.vector.tensor_scalar_mul(offs_f[:], offs_f[:], OFF_STEP)

            zero_t = consts.tile([128, (NUM_COEF - 1) * C], F32)
            nc.vector.memset(zero_t[:], 0.0)

            ident = consts.tile([128, 128], BF16)
            make_identity(nc, ident[:])

            # iota down partitions (value = partition index) as f32, for one-hots
            iota_p = consts.tile([128, 1], I32)
            nc.gpsimd.iota(iota_p[:], pattern=[[0, 1]], base=0, channel_multiplier=1)
            iota_pf = consts.tile([128, 1], F32)
            nc.vector.tensor_copy(iota_pf[:], iota_p[:])
            iota_pb = consts.tile([128, 1], BF16)
            nc.vector.tensor_copy(iota_pb[:], iota_p[:])

            # embedding tables resident in SBUF, split into bf16 hi+lo
            # (hi + lo == f32 value to ~2^-18 relative; matmul accumulates in f32)
            def split_table(dram_t, name):
                f = consts.tile([MAX_ELEM, C], F32, name=name + "_f")
                nc.sync.dma_start(out=f[:], in_=dram_t[:])
                hi = consts.tile([MAX_ELEM, C], BF16, name=name + "_hi")
                nc.vector.tensor_copy(hi[:], f[:])
                lo_f = consts.tile([MAX_ELEM, C], F32, name=name + "_lof")
                nc.vector.tensor_sub(lo_f[:], f[:], hi[:])
                lo = consts.tile([MAX_ELEM, C], BF16, name=name + "_lo")
                nc.vector.tensor_copy(lo[:], lo_f[:])
                return hi, lo

            srcw_hi, srcw_lo = split_table(source_w, "srcw")
            tgtw_hi, tgtw_lo = split_table(target_w, "tgtw")

            # ---- persistent edge-index tiles: idx32[p, ch] = idx[128*ch + p] ----
            s_i32 = consts.tile([128, E_CH], I32, name="s_i32")
            t_i32 = consts.tile([128, E_CH], I32, name="t_i32")
            nc.sync.dma_start(out=s_i32[:], in_=_ap(src_idx, 0, [[1, 128], [128, E_CH]]))
            nc.sync.dma_start(out=t_i32[:], in_=_ap(tgt_idx, 0, [[1, 128], [128, E_CH]]))

            # ---- packed node table [N_NODES, 4]: (x, y, z, float(an)) ----
            packed = dram.tile([N_NODES, 4], F32)
            nc.sync.dma_start(
                out=_ap(packed, packed.offset, [[4, N_NODES], [1, 3]]),
                in_=pos[:],
            )
            NACH = (N_NODES + 127) // 128  # 157
            an_sb = consts.tile([128, NACH], I32, name="an_sb")
            nc.vector.memset(an_sb[:], 0)
            nc.sync.dma_start(
                out=an_sb[:, :NACH - 1],
                in_=_ap(an_full, 0, [[1, 128], [128, NACH - 1]]),
            )
            nc.sync.dma_start(  # tail chunk (32 nodes)
                out=an_sb[0:N_NODES - 128 * (NACH - 1), NACH - 1:NACH],
                in_=_ap(an_full, 128 * (NACH - 1), [[1, N_NODES - 128 * (NACH - 1)], [1, 1]]),
            )
            an_f = consts.tile([128, NACH], F32, name="an_f")
            nc.vector.tensor_copy(an_f[:], an_sb[:])
            nc.sync.dma_start(
                out=_ap(packed, packed.offset + 3, [[4, 128], [512, NACH - 1], [1, 1]]),
                in_=an_f[:, :NACH - 1],
            )
            nc.sync.dma_start(
                out=_ap(packed, packed.offset + 3 + 512 * (NACH - 1),
                        [[4, N_NODES - 128 * (NACH - 1)], [1, 1]]),
                in_=an_f[0:N_NODES - 128 * (NACH - 1), NACH - 1:NACH],
            )

            # ---- SoA tiles for the rotation-matrix stage ----
            evx = consts.tile([128, E_CH], F32, name="evx")
            evy = consts.tile([128, E_CH], F32, name="evy")
            evz = consts.tile([128, E_CH], F32, name="evz")
            d_all = consts.tile([128, E_CH], F32, name="d_all")

            sg_bounds = []
            ch0 = 0
            while ch0 < E_CH:
                sg_bounds.append((ch0, min(SG_CH, E_CH - ch0)))
                ch0 += SG_CH

            with tc.tile_pool(name="sg", bufs=3) as sg, \
                 tc.tile_pool(name="ps", bufs=2, space="PSUM") as ps:
                for (c0, nch) in sg_bounds:
                    e0 = c0 * 128
                    # gather packed rows for src/tgt, one [128,1]-offset op per chunk
                    pk_src = sg.tile([128, SG_CH * 4], F32, tag="psrc", bufs=6)
                    pk_tgt = sg.tile([128, SG_CH * 4], F32, tag="ptgt", bufs=6)
                    for t in range(nch):
                        nc.gpsimd.indirect_dma_start(
                            out=pk_src[:, 4 * t:4 * t + 4], out_offset=None,
                            in_=packed[:],
                            in_offset=IndirectOffsetOnAxis(ap=s_i32[:, c0 + t:c0 + t + 1], axis=0),
                        )
                        nc.gpsimd.indirect_dma_start(
                            out=pk_tgt[:, 4 * t:4 * t + 4], out_offset=None,
                            in_=packed[:],
                            in_offset=IndirectOffsetOnAxis(ap=t_i32[:, c0 + t:c0 + t + 1], axis=0),
                        )
                    # ev components into SoA tiles
                    for comp, dst in ((0, evx), (1, evy), (2, evz)):
                        nc.vector.tensor_tensor(
                            out=dst[:, c0:c0 + nch],
                            in0=_ap(pk_src, pk_src.offset + comp, [pk_src.ap[0], [4, nch]]),
                            in1=_ap(pk_tgt, pk_tgt.offset + comp, [pk_tgt.ap[0], [4, nch]]),
                            op=OP.subtract,
                        )
                    # d = sqrt(evx^2 + evy^2 + evz^2)
                    t1 = sg.tile([128, SG_CH], F32, tag="t1", bufs=6)
                    t2 = sg.tile([128, SG_CH], F32, tag="t2", bufs=6)
                    nc.vector.tensor_mul(t1[:, :nch], evx[:, c0:c0 + nch], evx[:, c0:c0 + nch])
                    nc.vector.tensor_mul(t2[:, :nch], evy[:, c0:c0 + nch], evy[:, c0:c0 + nch])
                    nc.vector.tensor_add(t1[:, :nch], t1[:, :nch], t2[:, :nch])
                    nc.vector.tensor_mul(t2[:, :nch], evz[:, c0:c0 + nch], evz[:, c0:c0 + nch])
                    nc.vector.tensor_add(t1[:, :nch], t1[:, :nch], t2[:, :nch])
                    nc.scalar.activation(d_all[:, c0:c0 + nch], t1[:, :nch], AF.Sqrt)

                    # gaussian smearing -> feat_out[:, 0:128]
                    sm = sg.tile([128, SG_CH * NG], F32, tag="sm")
                    nc.vector.tensor_tensor(
                        out=sm[:, :nch * NG],
                        in0=_ap(d_all, d_all.offset + c0, [d_all.ap[0], [1, nch], [0, NG]]),
                        in1=_ap(offs_f, offs_f.offset, [offs_f.ap[0], [0, nch], [1, NG]]),
                        op=OP.subtract,
                    )
                    nc.vector.tensor_mul(sm[:, :nch * NG], sm[:, :nch * NG], sm[:, :nch * NG])
                    smear = sg.tile([128, SG_CH * NG], F32, tag="smear")
                    nc.scalar.activation(smear[:, :nch * NG], sm[:, :nch * NG], AF.Exp, scale=COEFF)
                    nc.sync.dma_start(
                        out=_ap(feat_out, e0 * 3 * NG,
                                [[3 * NG, 128], [3 * NG * 128, nch], [1, NG]]),
                        in_=_ap(smear, smear.offset, [smear.ap[0], [NG, nch], [1, NG]]),
                    )

                    # source/target embeddings via bf16 one-hot matmul -> feat_out[:, 128:384]
                    an_bf = sg.tile([128, SG_CH * 2], BF16, tag="anbf", bufs=6)
                    nc.vector.tensor_copy(
                        an_bf[:, 0:nch],
                        _ap(pk_src, pk_src.offset + 3, [pk_src.ap[0], [4, nch]]),
                    )
                    nc.vector.tensor_copy(
                        an_bf[:, SG_CH:SG_CH + nch],
                        _ap(pk_tgt, pk_tgt.offset + 3, [pk_tgt.ap[0], [4, nch]]),
                    )
                    emb = sg.tile([128, SG_CH * 2 * C], F32, tag="emb")
                    for t in range(nch):
                        pe = ps.tile([128, 2 * C], F32, tag="pe")
                        for side, (hi, lo) in enumerate(((srcw_hi, srcw_lo), (tgtw_hi, tgtw_lo))):
                            col = t if side == 0 else SG_CH + t
                            anb = _ap(an_bf, an_bf.offset + col, [an_bf.ap[0], [0, 128]])
                            pT = ps.tile([128, 128], BF16, tag=f"pT{side}")
                            nc.tensor.transpose(out=pT[:], in_=anb, identity=ident[:])
                            oh = sg.tile([MAX_ELEM, 128], BF16, tag=f"oh{side}")
                            nc.vector.tensor_tensor(
                                out=oh[:], in0=pT[0:MAX_ELEM, :],
                                in1=_ap(iota_pb, iota_pb.offset, [[iota_pb.ap[0][0], MAX_ELEM], [0, 128]]),
                                op=OP.is_equal,
                            )
                            nc.tensor.matmul(
                                out=pe[:, side * C:(side + 1) * C],
                                lhsT=oh[:], rhs=hi[:], start=True, stop=False,
                            )
                            nc.tensor.matmul(
                                out=pe[:, side * C:(side + 1) * C],
                                lhsT=oh[:], rhs=lo[:], start=False, stop=True,
                            )
                        nc.vector.tensor_copy(emb[:, 2 * C * t:2 * C * (t + 1)], pe[:])
                    nc.sync.dma_start(
                        out=_ap(feat_out, e0 * 3 * NG + NG,
                                [[3 * NG, 128], [3 * NG * 128, nch], [1, 2 * C]]),
                        in_=_ap(emb, emb.offset, [emb.ap[0], [2 * C, nch], [1, 2 * C]]),
                    )

            # ---- rotation matrices (SoA over [128, E_CH]) ----
            with tc.tile_pool(name="rot", bufs=1) as rp:
                rot_all = rp.tile([128, E_CH * 9], F32)
                HALF = (E_CH + 1) // 2
                for h0 in range(0, E_CH, HALF):
                    hw = min(HALF, E_CH - h0)

                    def rt(name):
                        return rp.tile([128, HALF], F32, name=f"{name}_{h0}", tag=name)[:, :hw]

                    sl = slice(h0, h0 + hw)
                    V = nc.vector
                    S = nc.scalar

                    nxx, nxy, nxz = rt("nxx"), rt("nxy"), rt("nxz")
                    rinv = rt("rinv")
                    V.reciprocal(rinv, d_all[:, sl])
                    V.tensor_mul(nxx, evx[:, sl], rinv)
                    V.tensor_mul(nxy, evy[:, sl], rinv)
                    V.tensor_mul(nxz, evz[:, sl], rinv)

                    e2x, e2y, e2z = rt("e2x"), rt("e2y"), rt("e2z")
                    for comp, dst in ((0, e2x), (1, e2y), (2, e2z)):
                        nc.sync.dma_start(
                            out=dst,
                            in_=_ap(rand, h0 * 128 * 3 + comp, [[3, 128], [384, hw]]),
                        )
                        V.tensor_scalar_add(dst, dst, -0.5)
                    s1, s2 = rt("s1"), rt("s2")
                    V.tensor_mul(s1, e2x, e2x)
                    V.tensor_mul(s2, e2y, e2y)
                    V.tensor_add(s1, s1, s2)
                    V.tensor_mul(s2, e2z, e2z)
                    V.tensor_add(s1, s1, s2)
                    S.activation(s1, s1, AF.Sqrt)
                    V.reciprocal(s2, s1)
                    V.tensor_mul(e2x, e2x, s2)
                    V.tensor_mul(e2y, e2y, s2)
                    V.tensor_mul(e2z, e2z, s2)

                    def adot(ax, ay, az, out, tmp):
                        V.tensor_mul(out, ax, nxx)
                        V.tensor_mul(tmp, ay, nxy)
                        V.tensor_add(out, out, tmp)
                        V.tensor_mul(tmp, az, nxz)
                        V.tensor_add(out, out, tmp)
                        S.activation(out, out, AF.Abs)

                    e2bx, e2by, e2bz = rt("e2bx"), rt("e2by"), rt("e2bz")
                    V.tensor_scalar_mul(e2bx, e2y, -1.0)
                    V.tensor_copy(e2by, e2x)
                    V.tensor_copy(e2bz, e2z)
                    e2cx, e2cy, e2cz = rt("e2cx"), rt("e2cy"), rt("e2cz")
                    V.tensor_copy(e2cx, e2x)
                    V.tensor_scalar_mul(e2cy, e2z, -1.0)
                    V.tensor_copy(e2cz, e2y)

                    vd, vdb, vdc, tmp = rt("vd"), rt("vdb"), rt("vdc"), rt("tmp")
                    adot(e2x, e2y, e2z, vd, tmp)
                    adot(e2bx, e2by, e2bz, vdb, tmp)
                    adot(e2cx, e2cy, e2cz, vdc, tmp)

                    mask = rp.tile([128, HALF], I32, name=f"mask_{h0}", tag="mask")[:, :hw]
                    V.tensor_tensor(out=mask, in0=vd, in1=vdb, op=OP.is_gt)
                    V.select(e2x, mask, e2bx, e2x)
                    V.select(e2y, mask, e2by, e2y)
                    V.select(e2z, mask, e2bz, e2z)

                    adot(e2x, e2y, e2z, vd, tmp)
                    V.tensor_tensor(out=mask, in0=vd, in1=vdc, op=OP.is_gt)
                    V.select(e2x, mask, e2cx, e2x)
                    V.select(e2y, mask, e2cy, e2y)
                    V.select(e2z, mask, e2cz, e2z)

                    def cross(ax, ay, az, bx, by, bz, ox, oy, oz, tmp):
                        V.tensor_mul(ox, ay, bz)
                        V.tensor_mul(tmp, az, by)
                        V.tensor_sub(ox, ox, tmp)
                        V.tensor_mul(oy, az, bx)
                        V.tensor_mul(tmp, ax, bz)
                        V.tensor_sub(oy, oy, tmp)
                        V.tensor_mul(oz, ax, by)
                        V.tensor_mul(tmp, ay, bx)
                        V.tensor_sub(oz, oz, tmp)

                    def normalize(ax, ay, az, s1, s2):
                        V.tensor_mul(s1, ax, ax)
                        V.tensor_mul(s2, ay, ay)
                        V.tensor_add(s1, s1, s2)
                        V.tensor_mul(s2, az, az)
                        V.tensor_add(s1, s1, s2)
                        S.activation(s1, s1, AF.Sqrt)
                        V.reciprocal(s2, s1)
                        V.tensor_mul(ax, ax, s2)
                        V.tensor_mul(ay, ay, s2)
                        V.tensor_mul(az, az, s2)

                    nzx, nzy, nzz = rt("nzx"), rt("nzy"), rt("nzz")
                    cross(nxx, nxy, nxz, e2x, e2y, e2z, nzx, nzy, nzz, tmp)
                    normalize(nzx, nzy, nzz, s1, s2)
                    nyx, nyy, nyz = rt("nyx"), rt("nyy"), rt("nyz")
                    cross(nxx, nxy, nxz, nzx, nzy, nzz, nyx, nyy, nyz, tmp)
                    normalize(nyx, nyy, nyz, s1, s2)

                    for j, src_t in enumerate((nzx, nzy, nzz, nxx, nxy, nxz, nyx, nyy, nyz)):
                        V.tensor_copy(
                            _ap(rot_all, rot_all.offset + h0 * 9 + j, [rot_all.ap[0], [9, hw]]),
                            src_t,
                        )
                nc.sync.dma_start(
                    out=_ap(rot_out, 0, [[9, 128], [9 * 128, E_CH], [1, 9]]),
                    in_=_ap(rot_all, rot_all.offset, [rot_all.ap[0], [9, E_CH], [1, 9]]),
                )

            # ---- node embeddings ----
            with tc.tile_pool(name="node", bufs=1) as npool:
                pw_i = npool.tile([128, 15], I32)
                nc.gpsimd.iota(pw_i[:], pattern=[[-1, 15]], base=14, channel_multiplier=0)
                one_i = npool.tile([128, 15], I32)
                nc.vector.memset(one_i[:], 1)
                nc.vector.tensor_tensor(out=pw_i[:], in0=one_i[:], in1=pw_i[:],
                                        op=mybir.AluOpType.logical_shift_left)
                pw_f = npool.tile([128, 15], F32)
                nc.vector.tensor_copy(pw_f[:], pw_i[:])

                an_t = npool.tile([128, NP_CH], I32)
                nc.sync.dma_start(out=an_t[:], in_=_ap(an_node, 0, [[1, 128], [128, NP_CH]]))

                xb_i = npool.tile([128, NP_CH * 15], I32)
                nc.sync.dma_start(
                    out=xb_i[:],
                    in_=_ap(xbits, 0, [[15, 128], [15 * 128, NP_CH], [1, 15]]),
                )
                xb_f = npool.tile([128, NP_CH * 15], F32)
                nc.vector.tensor_copy(xb_f[:], xb_i[:])
                prod = npool.tile([128, NP_CH * 15], F32)
                nc.vector.tensor_tensor(
                    out=prod[:],
                    in0=xb_f[:],
                    in1=_ap(pw_f, pw_f.offset, [pw_f.ap[0], [0, NP_CH], [1, 15]]),
                    op=mybir.AluOpType.mult,
                )
                xtra_f = npool.tile([128, NP_CH], F32)
                nc.vector.tensor_reduce(
                    out=xtra_f[:],
                    in_=_ap(prod, prod.offset, [prod.ap[0], [15, NP_CH], [1, 15]]),
                    axis=mybir.AxisListType.X,
                    op=mybir.AluOpType.add,
                )
                nc.vector.tensor_scalar_min(xtra_f[:], xtra_f[:], float(EMB2 - 1))
                xtra_i = npool.tile([128, NP_CH], I32)
                nc.vector.tensor_copy(xtra_i[:], xtra_f[:])

                g1 = npool.tile([128, NP_CH * C], F32)
                g2 = npool.tile([128, NP_CH * C], F32)
                for t in range(NP_CH):
                    nc.gpsimd.indirect_dma_start(
                        out=g1[:, C * t:C * (t + 1)], out_offset=None, in_=sphere_w[:],
                        in_offset=IndirectOffsetOnAxis(ap=an_t[:, t:t + 1], axis=0),
                    )
                    nc.gpsimd.indirect_dma_start(
                        out=g2[:, C * t:C * (t + 1)], out_offset=None, in_=sphere2_w[:],
                        in_offset=IndirectOffsetOnAxis(ap=xtra_i[:, t:t + 1], axis=0),
                    )
                nc.vector.tensor_add(g1[:], g1[:], g2[:])
                nc.sync.dma_start(
                    out=_ap(node_out, 0,
                            [[NUM_COEF * C, 128], [NUM_COEF * C * 128, NP_CH], [1, C]]),
                    in_=_ap(g1, g1.offset, [g1.ap[0], [C, NP_CH], [1, C]]),
                )
                nc.sync.dma_start(
                    out=_ap(node_out, C,
                            [[NUM_COEF * C, 128], [NUM_COEF * C * 128, NP_CH],
                             [1, (NUM_COEF - 1) * C]]),
                    in_=_ap(zero_t, zero_t.offset,
                            [zero_t.ap[0], [0, NP_CH], [1, (NUM_COEF - 1) * C]]),
                )

    nc.compile()
    return nc


_NC = None


def kernel(**inputs):
    global _NC, LAST_RESULT
    if _NC is None:
        _NC = build_module()
    nc = _NC

    pos = np.ascontiguousarray(np.asarray(inputs["pos"], dtype=np.float32))
    sphere_w = np.ascontiguousarray(np.asarray(inputs["sphere_w"], dtype=np.float32))
    sphere2_w = np.ascontiguousarray(np.asarray(inputs["sphere2_w"], dtype=np.float32))
    source_w = np.ascontiguousarray(np.asarray(inputs["source_w"], dtype=np.float32))
    target_w = np.ascontiguousarray(np.asarray(inputs["target_w"], dtype=np.float32))
    rand_vec = np.asarray(inputs["rand_vec"], dtype=np.float32)
    an = np.asarray(inputs["atomic_numbers"]).astype(np.int32)
    x_bits = np.asarray(inputs["x_bits"]).astype(np.int32)
    edge_index = np.asarray(inputs["edge_index"]).astype(np.int32)

    in_maps = []
    for c in range(N_CORES):
        e0 = c * E_PER
        src = np.zeros(E_PAD, np.int32)
        tgt = np.zeros(E_PAD, np.int32)
        src[:E_PER] = edge_index[0, e0:e0 + E_PER]
        tgt[:E_PER] = edge_index[1, e0:e0 + E_PER]
        tgt[E_PER:] = 1  # distinct from src pad (0) so padded edges stay finite
        rnd = np.full((E_PAD, 3), 0.25, np.float32)
        rnd[:E_PER] = rand_vec[e0:e0 + E_PER]
        n0 = c * NP_PER
        an_n = np.zeros(NP_PAD, np.int32)
        an_n[:NP_PER] = an[n0:n0 + NP_PER]
        xb = np.zeros((NP_PAD, 15), np.int32)
        xb[:NP_PER] = x_bits[n0:n0 + NP_PER]
        in_maps.append({
            "pos": pos, "sphere_w": sphere_w, "sphere2_w": sphere2_w,
            "source_w": source_w, "target_w": target_w, "an_full": an,
            "src_idx": np.ascontiguousarray(src), "tgt_idx": np.ascontiguousarray(tgt),
            "rand": np.ascontiguousarray(rnd), "an_node": an_n,
            "xbits": np.ascontiguousarray(xb),
        })

    res = run_bass_kernel_spmd(nc, in_maps, core_ids=list(range(N_CORES)), trace=TRACE)
    LAST_RESULT = res

    node_emb = np.concatenate([res.results[c]["node_out"][:NP_PER] for c in range(N_CORES)], axis=0)
    edge_feat = np.concatenate([res.results[c]["feat_out"][:E_PER] for c in range(N_CORES)], axis=0)
    rot = np.concatenate([res.results[c]["rot_out"][:E_PER] for c in range(N_CORES)], axis=0)
    return node_emb, edge_feat, rot.reshape(N_EDGES, 3, 3)


# revision 11
# speedup vs baseline: 1.0357x; 1.0156x over previous
"""Trainium2 Bass kernel for nn_EquivariantEmbedding (gnn_message_passing).

Sharding: edge-parallel across 8 cores (58500 edges each, padded to 58624);
node embedding data-parallel (2500 nodes/core, padded to 2560). Small
embedding tables are replicated. All compute (gathers, rotation matrices,
gaussian smearing, source/target one-hot embeddings, bit-packing) happens
on device.

Per-edge node data (pos + atomic number, packed 16B rows) is gathered with
SWDGE indirect DMAs, 128 rows per op. Source/target embeddings are computed
on the tensor engine as one-hot(atomic number) @ table matmuls — no 512B-row
gathers needed. Rotation matrices are computed in a structure-of-arrays
layout with full-width vector ops.
"""
import sys

for _p in ("/opt/trn_rl_repo",):
    if _p not in sys.path:
        sys.path.append(_p)

import numpy as np

import concourse.bacc as bacc
import concourse.bass as bass
import concourse.tile as tile
from concourse import mybir
from concourse.bass import IndirectOffsetOnAxis
from concourse.bass_utils import run_bass_kernel_spmd
from concourse.masks import make_identity

F32 = mybir.dt.float32
BF16 = mybir.dt.bfloat16
I32 = mybir.dt.int32

N_CORES = 8
N_NODES = 20000
N_EDGES = 468000
C = 128
NG = 128
NUM_COEF = 16
MAX_ELEM = 90
EMB2 = 32767
CUTOFF = 5.0
BWS = 20.0

E_PER = N_EDGES // N_CORES            # 58500
E_CH = (E_PER + 127) // 128           # 458 chunks of 128 edges
E_PAD = E_CH * 128                    # 58624

NP_PER = N_NODES // N_CORES           # 2500
NP_CH = (NP_PER + 127) // 128         # 20
NP_PAD = NP_CH * 128                  # 2560

SG_CH = 16                            # chunks per supergroup (2048 edges)

_offs = np.linspace(0.0, CUTOFF, NG, dtype=np.float32)
OFF_STEP = float(_offs[1] - _offs[0])
COEFF = float(-0.5 / (BWS * (_offs[1] - _offs[0])) ** 2)

TRACE = False  # test.py sets this for profiled runs
LAST_RESULT = None


def _ap(t, offset, dims):
    return bass.AP(tensor=t.tensor if hasattr(t, "tensor") else t, offset=offset,
                   ap=[list(d) for d in dims])


def build_module():
    nc = bacc.Bacc("TRN2", target_bir_lowering=False, debug=False, num_devices=N_CORES)

    pos = nc.dram_tensor("pos", [N_NODES, 3], F32, kind="ExternalInput")
    sphere_w = nc.dram_tensor("sphere_w", [MAX_ELEM, C], F32, kind="ExternalInput")
    sphere2_w = nc.dram_tensor("sphere2_w", [EMB2, C], F32, kind="ExternalInput")
    source_w = nc.dram_tensor("source_w", [MAX_ELEM, NG], F32, kind="ExternalInput")
    target_w = nc.dram_tensor("target_w", [MAX_ELEM, NG], F32, kind="ExternalInput")
    an_full = nc.dram_tensor("an_full", [N_NODES], I32, kind="ExternalInput")
    src_idx = nc.dram_tensor("src_idx", [E_PAD], I32, kind="ExternalInput")
    tgt_idx = nc.dram_tensor("tgt_idx", [E_PAD], I32, kind="ExternalInput")
    rand = nc.dram_tensor("rand", [E_PAD, 3], F32, kind="ExternalInput")
    an_node = nc.dram_tensor("an_node", [NP_PAD], I32, kind="ExternalInput")
    xbits = nc.dram_tensor("xbits", [NP_PAD, 15], I32, kind="ExternalInput")

    node_out = nc.dram_tensor("node_out", [NP_PAD, NUM_COEF, C], F32, kind="ExternalOutput")
    feat_out = nc.dram_tensor("feat_out", [E_PAD, 3 * NG], F32, kind="ExternalOutput")
    rot_out = nc.dram_tensor("rot_out", [E_PAD, 9], F32, kind="ExternalOutput")

    AF = mybir.ActivationFunctionType
    OP = mybir.AluOpType

    with tile.TileContext(nc) as tc:
        import contextlib
        with contextlib.ExitStack() as ctx:
            consts = ctx.enter_context(tc.tile_pool(name="consts", bufs=1))
            dram = ctx.enter_context(tc.tile_pool(name="dram", bufs=1, space="DRAM"))

            # ---- constants ----
            offs_i = consts.tile([128, NG], I32)
            nc.gpsimd.iota(offs_i[:], pattern=[[1, NG]], base=0, channel_multiplier=0)
            offs_f = consts.tile([128, NG], F32)
            nc.vector.tensor_copy(offs_f[:], offs_i[:])
            nc.vector.tensor_scalar_mul(offs_f[:], offs_f[:], OFF_STEP)

            zero_t = consts.tile([128, (NUM_COEF - 1) * C], F32)
            nc.vector.memset(zero_t[:], 0.0)

            ident = consts.tile([128, 128], BF16)
            make_identity(nc, ident[:])

            # iota down partitions (value = partition index) as f32, for one-hots
            iota_p = consts.tile([128, 1], I32)
            nc.gpsimd.iota(iota_p[:], pattern=[[0, 1]], base=0, channel_multiplier=1)
            iota_pf = consts.tile([128, 1], F32)
            nc.vector.tensor_copy(iota_pf[:], iota_p[:])
            iota_pb = consts.tile([128, 1], BF16)
            nc.vector.tensor_copy(iota_pb[:], iota_p[:])

            # embedding tables resident in SBUF, split into bf16 hi+lo
            # (hi + lo == f32 value to ~2^-18 relative; matmul accumulates in f32)
            def split_table(dram_t, name):
                f = consts.tile([MAX_ELEM, C], F32, name=name + "_f")
                nc.sync.dma_start(out=f[:], in_=dram_t[:])
                hi = consts.tile([MAX_ELEM, C], BF16, name=name + "_hi")
                nc.vector.tensor_copy(hi[:], f[:])
                lo_f = consts.tile([MAX_ELEM, C], F32, name=name + "_lof")
                nc.vector.tensor_sub(lo_f[:], f[:], hi[:])
                lo = consts.tile([MAX_ELEM, C], BF16, name=name + "_lo")
                nc.vector.tensor_copy(lo[:], lo_f[:])
                return hi, lo

            srcw_hi, srcw_lo = split_table(source_w, "srcw")
            tgtw_hi, tgtw_lo = split_table(target_w, "tgtw")

            # ---- persistent edge-index tiles: idx32[p, ch] = idx[128*ch + p] ----
            s_i32 = consts.tile([128, E_CH], I32, name="s_i32")
            t_i32 = consts.tile([128, E_CH], I32, name="t_i32")
            nc.sync.dma_start(out=s_i32[:], in_=_ap(src_idx, 0, [[1, 128], [128, E_CH]]))
            nc.sync.dma_start(out=t_i32[:], in_=_ap(tgt_idx, 0, [[1, 128], [128, E_CH]]))

            # ---- packed node table [N_NODES, 4]: (x, y, z, float(an)) ----
            packed = dram.tile([N_NODES, 4], F32)
            nc.sync.dma_start(
                out=_ap(packed, packed.offset, [[4, N_NODES], [1, 3]]),
                in_=pos[:],
            )
            NACH = (N_NODES + 127) // 128  # 157
            an_sb = consts.tile([128, NACH], I32, name="an_sb")
            nc.vector.memset(an_sb[:], 0)
            nc.sync.dma_start(
                out=an_sb[:, :NACH - 1],
                in_=_ap(an_full, 0, [[1, 128], [128, NACH - 1]]),
            )
            nc.sync.dma_start(  # tail chunk (32 nodes)
                out=an_sb[0:N_NODES - 128 * (NACH - 1), NACH - 1:NACH],
                in_=_ap(an_full, 128 * (NACH - 1), [[1, N_NODES - 128 * (NACH - 1)], [1, 1]]),
            )
            an_f = consts.tile([128, NACH], F32, name="an_f")
            nc.vector.tensor_copy(an_f[:], an_sb[:])
            nc.sync.dma_start(
                out=_ap(packed, packed.offset + 3, [[4, 128], [512, NACH - 1], [1, 1]]),
                in_=an_f[:, :NACH - 1],
            )
            nc.sync.dma_start(
                out=_ap(packed, packed.offset + 3 + 512 * (NACH - 1),
                        [[4, N_NODES - 128 * (NACH - 1)], [1, 1]]),
                in_=an_f[0:N_NODES - 128 * (NACH - 1), NACH - 1:NACH],
            )

            # ---- node embeddings ----
            with tc.tile_pool(name="node", bufs=1) as npool:
                pw_i = npool.tile([128, 15], I32)
                nc.gpsimd.iota(pw_i[:], pattern=[[-1, 15]], base=14, channel_multiplier=0)
                one_i = npool.tile([128, 15], I32)
                nc.vector.memset(one_i[:], 1)
                nc.vector.tensor_tensor(out=pw_i[:], in0=one_i[:], in1=pw_i[:],
                                        op=mybir.AluOpType.logical_shift_left)
                pw_f = npool.tile([128, 15], F32)
                nc.vector.tensor_copy(pw_f[:], pw_i[:])

                an_t = npool.tile([128, NP_CH], I32)
                nc.sync.dma_start(out=an_t[:], in_=_ap(an_node, 0, [[1, 128], [128, NP_CH]]))

                xb_i = npool.tile([128, NP_CH * 15], I32)
                nc.sync.dma_start(
                    out=xb_i[:],
                    in_=_ap(xbits, 0, [[15, 128], [15 * 128, NP_CH], [1, 15]]),
                )
                xb_f = npool.tile([128, NP_CH * 15], F32)
                nc.vector.tensor_copy(xb_f[:], xb_i[:])
                prod = npool.tile([128, NP_CH * 15], F32)
                nc.vector.tensor_tensor(
                    out=prod[:],
                    in0=xb_f[:],
                    in1=_ap(pw_f, pw_f.offset, [pw_f.ap[0], [0, NP_CH], [1, 15]]),
                    op=mybir.AluOpType.mult,
                )
                xtra_f = npool.tile([128, NP_CH], F32)
                nc.vector.tensor_reduce(
                    out=xtra_f[:],
                    in_=_ap(prod, prod.offset, [prod.ap[0], [15, NP_CH], [1, 15]]),
                    axis=mybir.AxisListType.X,
                    op=mybir.AluOpType.add,
                )
                nc.vector.tensor_scalar_min(xtra_f[:], xtra_f[:], float(EMB2 - 1))
                xtra_i = npool.tile([128, NP_CH], I32)
                nc.vector.tensor_copy(xtra_i[:], xtra_f[:])

                g1 = npool.tile([128, NP_CH * C], F32)
                g2 = npool.tile([128, NP_CH * C], F32)
                for t in range(NP_CH):
                    nc.gpsimd.indirect_dma_start(
                        out=g1[:, C * t:C * (t + 1)], out_offset=None, in_=sphere_w[:],
                        in_offset=IndirectOffsetOnAxis(ap=an_t[:, t:t + 1], axis=0),
                    )
                    nc.gpsimd.indirect_dma_start(
                        out=g2[:, C * t:C * (t + 1)], out_offset=None, in_=sphere2_w[:],
                        in_offset=IndirectOffsetOnAxis(ap=xtra_i[:, t:t + 1], axis=0),
                    )
                nc.vector.tensor_add(g1[:], g1[:], g2[:])
                nc.sync.dma_start(
                    out=_ap(node_out, 0,
                            [[NUM_COEF * C, 128], [NUM_COEF * C * 128, NP_CH], [1, C]]),
                    in_=_ap(g1, g1.offset, [g1.ap[0], [C, NP_CH], [1, C]]),
                )
                nc.sync.dma_start(
                    out=_ap(node_out, C,
                            [[NUM_COEF * C, 128], [NUM_COEF * C * 128, NP_CH],
                             [1, (NUM_COEF - 1) * C]]),
                    in_=_ap(zero_t, zero_t.offset,
                            [zero_t.ap[0], [0, NP_CH], [1, (NUM_COEF - 1) * C]]),
                )


            # ---- SoA tiles for the rotation-matrix stage ----
            evx = consts.tile([128, E_CH], F32, name="evx")
            evy = consts.tile([128, E_CH], F32, name="evy")
            evz = consts.tile([128, E_CH], F32, name="evz")
            d_all = consts.tile([128, E_CH], F32, name="d_all")

            sg_bounds = []
            ch0 = 0
            while ch0 < E_CH:
                sg_bounds.append((ch0, min(SG_CH, E_CH - ch0)))
                ch0 += SG_CH

            with tc.tile_pool(name="sg", bufs=3) as sg, \
                 tc.tile_pool(name="ps", bufs=2, space="PSUM") as ps:
                for (c0, nch) in sg_bounds:
                    e0 = c0 * 128
                    # gather packed rows for src/tgt, one [128,1]-offset op per chunk
                    pk_src = sg.tile([128, SG_CH * 4], F32, tag="psrc", bufs=12)
                    pk_tgt = sg.tile([128, SG_CH * 4], F32, tag="ptgt", bufs=12)
                    for t in range(nch):
                        nc.gpsimd.indirect_dma_start(
                            out=pk_src[:, 4 * t:4 * t + 4], out_offset=None,
                            in_=packed[:],
                            in_offset=IndirectOffsetOnAxis(ap=s_i32[:, c0 + t:c0 + t + 1], axis=0),
                        )
                        nc.gpsimd.indirect_dma_start(
                            out=pk_tgt[:, 4 * t:4 * t + 4], out_offset=None,
                            in_=packed[:],
                            in_offset=IndirectOffsetOnAxis(ap=t_i32[:, c0 + t:c0 + t + 1], axis=0),
                        )
                    # ev components into SoA tiles
                    for comp, dst in ((0, evx), (1, evy), (2, evz)):
                        nc.vector.tensor_tensor(
                            out=dst[:, c0:c0 + nch],
                            in0=_ap(pk_src, pk_src.offset + comp, [pk_src.ap[0], [4, nch]]),
                            in1=_ap(pk_tgt, pk_tgt.offset + comp, [pk_tgt.ap[0], [4, nch]]),
                            op=OP.subtract,
                        )
                    # d = sqrt(evx^2 + evy^2 + evz^2)
                    t1 = sg.tile([128, SG_CH], F32, tag="t1", bufs=8)
                    t2 = sg.tile([128, SG_CH], F32, tag="t2", bufs=8)
                    nc.vector.tensor_mul(t1[:, :nch], evx[:, c0:c0 + nch], evx[:, c0:c0 + nch])
                    nc.vector.tensor_mul(t2[:, :nch], evy[:, c0:c0 + nch], evy[:, c0:c0 + nch])
                    nc.vector.tensor_add(t1[:, :nch], t1[:, :nch], t2[:, :nch])
                    nc.vector.tensor_mul(t2[:, :nch], evz[:, c0:c0 + nch], evz[:, c0:c0 + nch])
                    nc.vector.tensor_add(t1[:, :nch], t1[:, :nch], t2[:, :nch])
                    nc.scalar.activation(d_all[:, c0:c0 + nch], t1[:, :nch], AF.Sqrt)

                    # gaussian smearing -> feat_out[:, 0:128]
                    sm = sg.tile([128, SG_CH * NG], F32, tag="sm")
                    nc.vector.tensor_tensor(
                        out=sm[:, :nch * NG],
                        in0=_ap(d_all, d_all.offset + c0, [d_all.ap[0], [1, nch], [0, NG]]),
                        in1=_ap(offs_f, offs_f.offset, [offs_f.ap[0], [0, nch], [1, NG]]),
                        op=OP.subtract,
                    )
                    nc.vector.tensor_mul(sm[:, :nch * NG], sm[:, :nch * NG], sm[:, :nch * NG])
                    smear = sg.tile([128, SG_CH * NG], F32, tag="smear")
                    nc.scalar.activation(smear[:, :nch * NG], sm[:, :nch * NG], AF.Exp, scale=COEFF)
                    nc.sync.dma_start(
                        out=_ap(feat_out, e0 * 3 * NG,
                                [[3 * NG, 128], [3 * NG * 128, nch], [1, NG]]),
                        in_=_ap(smear, smear.offset, [smear.ap[0], [NG, nch], [1, NG]]),
                    )

                    # source/target embeddings via bf16 one-hot matmul -> feat_out[:, 128:384]
                    an_bf = sg.tile([128, SG_CH * 2], BF16, tag="anbf", bufs=8)
                    nc.vector.tensor_copy(
                        an_bf[:, 0:nch],
                        _ap(pk_src, pk_src.offset + 3, [pk_src.ap[0], [4, nch]]),
                    )
                    nc.vector.tensor_copy(
                        an_bf[:, SG_CH:SG_CH + nch],
                        _ap(pk_tgt, pk_tgt.offset + 3, [pk_tgt.ap[0], [4, nch]]),
                    )
                    emb = sg.tile([128, SG_CH * 2 * C], F32, tag="emb")
                    for t in range(nch):
                        pe = ps.tile([128, 2 * C], F32, tag="pe")
                        for side, (hi, lo) in enumerate(((srcw_hi, srcw_lo), (tgtw_hi, tgtw_lo))):
                            col = t if side == 0 else SG_CH + t
                            anb = _ap(an_bf, an_bf.offset + col, [an_bf.ap[0], [0, 128]])
                            pT = ps.tile([128, 128], BF16, tag=f"pT{side}")
                            nc.tensor.transpose(out=pT[:], in_=anb, identity=ident[:])
                            oh = sg.tile([MAX_ELEM, 128], BF16, tag=f"oh{side}")
                            nc.vector.tensor_tensor(
                                out=oh[:], in0=pT[0:MAX_ELEM, :],
                                in1=_ap(iota_pb, iota_pb.offset, [[iota_pb.ap[0][0], MAX_ELEM], [0, 128]]),
                                op=OP.is_equal,
                            )
                            nc.tensor.matmul(
                                out=pe[:, side * C:(side + 1) * C],
                                lhsT=oh[:], rhs=hi[:], start=True, stop=False,
                            )
                            nc.tensor.matmul(
                                out=pe[:, side * C:(side + 1) * C],
                                lhsT=oh[:], rhs=lo[:], start=False, stop=True,
                            )
                        nc.vector.tensor_copy(emb[:, 2 * C * t:2 * C * (t + 1)], pe[:])
                    nc.sync.dma_start(
                        out=_ap(feat_out, e0 * 3 * NG + NG,
                                [[3 * NG, 128], [3 * NG * 128, nch], [1, 2 * C]]),
                        in_=_ap(emb, emb.offset, [emb.ap[0], [2 * C, nch], [1, 2 * C]]),
                    )

            # ---- rotation matrices (SoA over [128, E_CH]) ----
            with tc.tile_pool(name="rot", bufs=1) as rp:
                rot_all = rp.tile([128, E_CH * 9], F32)
                HALF = (E_CH + 1) // 2
                for h0 in range(0, E_CH, HALF):
                    hw = min(HALF, E_CH - h0)

                    def rt(name):
                        return rp.tile([128, HALF], F32, name=f"{name}_{h0}", tag=name)[:, :hw]

                    sl = slice(h0, h0 + hw)
                    V = nc.vector
                    S = nc.scalar

                    nxx, nxy, nxz = rt("nxx"), rt("nxy"), rt("nxz")
                    rinv = rt("rinv")
                    V.reciprocal(rinv, d_all[:, sl])
                    V.tensor_mul(nxx, evx[:, sl], rinv)
                    V.tensor_mul(nxy, evy[:, sl], rinv)
                    V.tensor_mul(nxz, evz[:, sl], rinv)

                    e2x, e2y, e2z = rt("e2x"), rt("e2y"), rt("e2z")
                    for comp, dst in ((0, e2x), (1, e2y), (2, e2z)):
                        nc.sync.dma_start(
                            out=dst,
                            in_=_ap(rand, h0 * 128 * 3 + comp, [[3, 128], [384, hw]]),
                        )
                        V.tensor_scalar_add(dst, dst, -0.5)
                    s1, s2 = rt("s1"), rt("s2")
                    V.tensor_mul(s1, e2x, e2x)
                    V.tensor_mul(s2, e2y, e2y)
                    V.tensor_add(s1, s1, s2)
                    V.tensor_mul(s2, e2z, e2z)
                    V.tensor_add(s1, s1, s2)
                    S.activation(s1, s1, AF.Sqrt)
                    V.reciprocal(s2, s1)
                    V.tensor_mul(e2x, e2x, s2)
                    V.tensor_mul(e2y, e2y, s2)
                    V.tensor_mul(e2z, e2z, s2)

                    def adot(ax, ay, az, out, tmp):
                        V.tensor_mul(out, ax, nxx)
                        V.tensor_mul(tmp, ay, nxy)
                        V.tensor_add(out, out, tmp)
                        V.tensor_mul(tmp, az, nxz)
                        V.tensor_add(out, out, tmp)
                        S.activation(out, out, AF.Abs)

                    e2bx, e2by, e2bz = rt("e2bx"), rt("e2by"), rt("e2bz")
                    V.tensor_scalar_mul(e2bx, e2y, -1.0)
                    V.tensor_copy(e2by, e2x)
                    V.tensor_copy(e2bz, e2z)
                    e2cx, e2cy, e2cz = rt("e2cx"), rt("e2cy"), rt("e2cz")
                    V.tensor_copy(e2cx, e2x)
                    V.tensor_scalar_mul(e2cy, e2z, -1.0)
                    V.tensor_copy(e2cz, e2y)

                    vd, vdb, vdc, tmp = rt("vd"), rt("vdb"), rt("vdc"), rt("tmp")
                    adot(e2x, e2y, e2z, vd, tmp)
                    adot(e2bx, e2by, e2bz, vdb, tmp)
                    adot(e2cx, e2cy, e2cz, vdc, tmp)

                    mask = rp.tile([128, HALF], I32, name=f"mask_{h0}", tag="mask")[:, :hw]
                    V.tensor_tensor(out=mask, in0=vd, in1=vdb, op=OP.is_gt)
                    V.select(e2x, mask, e2bx, e2x)
                    V.select(e2y, mask, e2by, e2y)
                    V.select(e2z, mask, e2bz, e2z)

                    adot(e2x, e2y, e2z, vd, tmp)
                    V.tensor_tensor(out=mask, in0=vd, in1=vdc, op=OP.is_gt)
                    V.select(e2x, mask, e2cx, e2x)
                    V.select(e2y, mask, e2cy, e2y)
                    V.select(e2z, mask, e2cz, e2z)

                    def cross(ax, ay, az, bx, by, bz, ox, oy, oz, tmp):
                        V.tensor_mul(ox, ay, bz)
                        V.tensor_mul(tmp, az, by)
                        V.tensor_sub(ox, ox, tmp)
                        V.tensor_mul(oy, az, bx)
                        V.tensor_mul(tmp, ax, bz)
                        V.tensor_sub(oy, oy, tmp)
                        V.tensor_mul(oz, ax, by)
                        V.tensor_mul(tmp, ay, bx)
                        V.tensor_sub(oz, oz, tmp)

                    def normalize(ax, ay, az, s1, s2):
                        V.tensor_mul(s1, ax, ax)
                        V.tensor_mul(s2, ay, ay)
                        V.tensor_add(s1, s1, s2)
                        V.tensor_mul(s2, az, az)
                        V.tensor_add(s1, s1, s2)
                        S.activation(s1, s1, AF.Sqrt)
                        V.reciprocal(s2, s1)
                        V.tensor_mul(ax, ax, s2)
                        V.tensor_mul(ay, ay, s2)
                        V.tensor_mul(az, az, s2)

                    nzx, nzy, nzz = rt("nzx"), rt("nzy"), rt("nzz")
                    cross(nxx, nxy, nxz, e2x, e2y, e2z, nzx, nzy, nzz, tmp)
                    normalize(nzx, nzy, nzz, s1, s2)
                    nyx, nyy, nyz = rt("nyx"), rt("nyy"), rt("nyz")
                    cross(nxx, nxy, nxz, nzx, nzy, nzz, nyx, nyy, nyz, tmp)
                    normalize(nyx, nyy, nyz, s1, s2)

                    for j, src_t in enumerate((nzx, nzy, nzz, nxx, nxy, nxz, nyx, nyy, nyz)):
                        V.tensor_copy(
                            _ap(rot_all, rot_all.offset + h0 * 9 + j, [rot_all.ap[0], [9, hw]]),
                            src_t,
                        )
                nc.sync.dma_start(
                    out=_ap(rot_out, 0, [[9, 128], [9 * 128, E_CH], [1, 9]]),
                    in_=_ap(rot_all, rot_all.offset, [rot_all.ap[0], [9, E_CH], [1, 9]]),
                )

    nc.compile()
    return nc


_NC = None


def kernel(**inputs):
    global _NC, LAST_RESULT
    if _NC is None:
        _NC = build_module()
    nc = _NC

    pos = np.ascontiguousarray(np.asarray(inputs["pos"], dtype=np.float32))
    sphere_w = np.ascontiguousarray(np.asarray(inputs["sphere_w"], dtype=np.float32))
    sphere2_w = np.ascontiguousarray(np.asarray(inputs["sphere2_w"], dtype=np.float32))
    source_w = np.ascontiguousarray(np.asarray(inputs["source_w"], dtype=np.float32))
    target_w = np.ascontiguousarray(np.asarray(inputs["target_w"], dtype=np.float32))
    rand_vec = np.asarray(inputs["rand_vec"], dtype=np.float32)
    an = np.asarray(inputs["atomic_numbers"]).astype(np.int32)
    x_bits = np.asarray(inputs["x_bits"]).astype(np.int32)
    edge_index = np.asarray(inputs["edge_index"]).astype(np.int32)

    in_maps = []
    for c in range(N_CORES):
        e0 = c * E_PER
        src = np.zeros(E_PAD, np.int32)
        tgt = np.zeros(E_PAD, np.int32)
        src[:E_PER] = edge_index[0, e0:e0 + E_PER]
        tgt[:E_PER] = edge_index[1, e0:e0 + E_PER]
        tgt[E_PER:] = 1  # distinct from src pad (0) so padded edges stay finite
        rnd = np.full((E_PAD, 3), 0.25, np.float32)
        rnd[:E_PER] = rand_vec[e0:e0 + E_PER]
        n0 = c * NP_PER
        an_n = np.zeros(NP_PAD, np.int32)
        an_n[:NP_PER] = an[n0:n0 + NP_PER]
        xb = np.zeros((NP_PAD, 15), np.int32)
        xb[:NP_PER] = x_bits[n0:n0 + NP_PER]
        in_maps.append({
            "pos": pos, "sphere_w": sphere_w, "sphere2_w": sphere2_w,
            "source_w": source_w, "target_w": target_w, "an_full": an,
            "src_idx": np.ascontiguousarray(src), "tgt_idx": np.ascontiguousarray(tgt),
            "rand": np.ascontiguousarray(rnd), "an_node": an_n,
            "xbits": np.ascontiguousarray(xb),
        })

    res = run_bass_kernel_spmd(nc, in_maps, core_ids=list(range(N_CORES)), trace=TRACE)
    LAST_RESULT = res

    node_emb = np.concatenate([res.results[c]["node_out"][:NP_PER] for c in range(N_CORES)], axis=0)
    edge_feat = np.concatenate([res.results[c]["feat_out"][:E_PER] for c in range(N_CORES)], axis=0)
    rot = np.concatenate([res.results[c]["rot_out"][:E_PER] for c in range(N_CORES)], axis=0)
    return node_emb, edge_feat, rot.reshape(N_EDGES, 3, 3)
